# revision 4
# baseline (speedup 1.0000x reference)
"""Trainium2 Bass kernel for nn_AttentionBlock (sliding-window attention block).

Distribution: tensor-parallel over the 8 KV head groups (one group per core).
Each core computes qkv^T for its group (640 rows x 2048 tokens), windowed
attention for its 8 q-heads, and a partial output projection; host sums the
8 partials and adds x + b_out.

Device layout is feature-major: features on partitions, tokens on the free
dim.  Host pre-transposes and pre-rmsnorms x (bf16) and the weights.

Key structure (v2):
- x shipped pre-normalized (x*sqrt(H)/s) so qkv psum needs no per-token
  normalize; bias enters via a ones u-row.
- xb chunk-contiguous in DRAM ([NCH, P, KH, TC]) for fat DMA descriptors.
- softmax denominator: sink added on DVE (scalar_tensor_tensor), reciprocal
  on DVE, partition-broadcast on GpSimd -- no PE rank-1 matmuls.
- attention output tile is [128, 4, TC] with head h -> (kk=h%4, half=h//4)
  so the normalize multiply writes it directly (no layout copies); w_out is
  host-reordered to match.
- v-transpose tiles padded to [128, 128] so the prob@v matmul gets fast
  weight load; col 64 is the ones column for the denominator row.
- global software pipeline: the PE queue of window ch interleaves
  attention(ch), out-proj(ch-1) and qkv(ch+1), so the out projection is
  hidden inside the token loop instead of running as a tail phase.
- optional fp8 out-projection (DoubleRow) controlled by FP8_OUT.
"""

import contextlib
import math

import numpy as np
import ml_dtypes

import concourse.bass as bass
import concourse.mybir as mybir
import concourse.tile as tile
from concourse import bacc, bass_utils

# ---- problem config (hardcoded from the reference) ----
HIDDEN = 2880
HEAD_DIM = 64
N_HEADS = 64
N_KV = 8
Q_MULT = N_HEADS // N_KV  # 8
SLIDING_WINDOW = 128
ROPE_BASE = 150000.0
ROPE_SCALE = 32.0
NTK_ALPHA = 1.0
NTK_BETA = 32.0
INIT_CTX = 4096
RMS_EPS = 1e-5
SM_SCALE = 1.0 / math.sqrt(HEAD_DIM)
Q_DIM = N_HEADS * HEAD_DIM  # 4096
KV_DIM = N_KV * HEAD_DIM  # 512
B, T = 1, 2048

N_CORES = 8
P = 128
TC = 512  # token chunk
NCH = T // TC  # 4 chunks
NT = TC // P  # 4 token blocks per chunk
KH = 23  # hidden tiles: 22x128 + 1x64 (+u row)
W_SIZES = [128] * 22 + [65]  # last tile carries the bias row
QKV_ROWS = Q_MULT * HEAD_DIM + 2 * HEAD_DIM  # 640
QKV_M = QKV_ROWS // P  # 5
CD = 23  # out-proj c tiles: 22x128 + 1x64
C_SIZES = [128] * 22 + [64]

F32 = mybir.dt.float32
BF16 = mybir.dt.bfloat16
F8E4 = mybir.dt.float8e4
AF = mybir.ActivationFunctionType
ALU = mybir.AluOpType

FP8_OUT = False           # fp8 (DoubleRow) out-projection
ATTN_SCALE = 8.0 if FP8_OUT else 1.0
WO_SCALE = 64.0 if FP8_OUT else 1.0

_CACHE = {}


# ------------------------- host-side preparation -------------------------

def _rope_tables():
    d_half = HEAD_DIM // 2
    freq = ROPE_BASE ** (np.arange(0, HEAD_DIM, 2, dtype=np.float64) / HEAD_DIM)
    concentration = 0.1 * math.log(ROPE_SCALE) + 1.0
    low = d_half * math.log(INIT_CTX / (NTK_BETA * 2 * math.pi)) / math.log(ROPE_BASE)
    high = d_half * math.log(INIT_CTX / (NTK_ALPHA * 2 * math.pi)) / math.log(ROPE_BASE)
    interpolation = 1.0 / (ROPE_SCALE * freq)
    extrapolation = 1.0 / freq
    ramp = (np.arange(d_half, dtype=np.float64) - low) / (high - low)
    mask = 1.0 - np.clip(ramp, 0.0, 1.0)
    inv_freq = interpolation * (1.0 - mask) + extrapolation * mask
    pos = np.arange(T, dtype=np.float64)
    angles = pos[:, None] * inv_freq[None, :]  # [T, 32]
    cos = (np.cos(angles) * concentration).astype(np.float32)
    sin = (np.sin(angles) * concentration).astype(np.float32)
    return cos.T.copy(), sin.T.copy()  # [32, T]


def _perm64():
    # evens then odds within a 64-dim head
    return np.concatenate([np.arange(0, 64, 2), np.arange(1, 64, 2)])


def _host_prepare(x, norm_scale, w_qkv, b_qkv, sinks, w_out, b_out):
    x64 = x[0].astype(np.float64)
    s = np.sqrt((x64 * x64).sum(axis=1) + HIDDEN * RMS_EPS)  # [2048]
    xn = x64 * (math.sqrt(HIDDEN) / s)[:, None]  # pre-rmsnormed, ~N(0,1)
    xf = np.zeros((KH * P, T), dtype=np.float32)
    xf[:HIDDEN] = xn.T
    xf[HIDDEN] = 1.0  # u row (bias)
    # chunk-contiguous: [NCH, P, KH, TC]
    xb = np.ascontiguousarray(
        xf.reshape(KH, P, NCH, TC).transpose(2, 1, 0, 3)).astype(ml_dtypes.bfloat16)

    w_eff = (w_qkv * norm_scale[None, :]).astype(np.float64)
    b_eff = b_qkv.astype(np.float64).copy()
    # fold softmax scale into q rows (rope is a rotation; scale commutes)
    w_eff[:Q_DIM] *= SM_SCALE
    b_eff[:Q_DIM] *= SM_SCALE

    perm = _perm64()
    cosT, sinT = _rope_tables()
    sin_signed = np.concatenate([-sinT, sinT], axis=0)  # [64, T]

    # band masks for the two score blocks, repeated over 4 heads
    pidx = np.arange(P)[:, None]
    jidx = np.arange(P)[None, :]
    mask_a = np.tile((jidx >= pidx), (1, 4)).astype(ml_dtypes.bfloat16)  # [128, 512]
    mask_b = np.tile((jidx <= pidx), (1, 4)).astype(ml_dtypes.bfloat16)

    per_core = []
    for g in range(N_CORES):
        rows = []
        for h in range(Q_MULT):  # q heads of this group, rope-permuted
            base = (g * Q_MULT + h) * HEAD_DIM
            rows.append(base + perm)
        rows.append(Q_DIM + g * HEAD_DIM + perm)  # k head, rope-permuted
        rows.append(Q_DIM + KV_DIM + g * HEAD_DIM + np.arange(HEAD_DIM))  # v natural
        rows = np.concatenate(rows)
        wq_g = np.concatenate(
            [w_eff[rows].T, b_eff[rows][None, :]], axis=0
        ).astype(ml_dtypes.bfloat16)  # [2881, 640]

        # w_out columns for this group, reordered so attn row r of kk-tile j is:
        #   rows 0:64  -> head j      (hg0)
        #   rows 64:128-> head 4 + j  (hg1)
        wo_local = w_out[:, g * KV_DIM:(g + 1) * KV_DIM].T  # [512, 2880]
        order = []
        for kk in range(4):
            order.extend(range(kk * 64, kk * 64 + 64))
            order.extend(range((4 + kk) * 64, (4 + kk) * 64 + 64))
        wo_g = np.ascontiguousarray(wo_local[order])  # [512, 2880]
        if FP8_OUT:
            wo_g = np.clip(wo_g * WO_SCALE, -240, 240).astype(ml_dtypes.float8_e4m3)
        else:
            wo_g = wo_g.astype(ml_dtypes.bfloat16)

        sexp = np.exp2(sinks[g * Q_MULT:(g + 1) * Q_MULT]).astype(np.float64)
        srow = np.repeat(sexp / ATTN_SCALE, P).reshape(1, -1).astype(np.float32)
        per_core.append({
            "xb": xb,
            "wq": wq_g,
            "wo": wo_g,
            "srow": srow,  # [1, 1024] f32, sink/ATTN_SCALE per head
            "cosT": cosT.astype(ml_dtypes.bfloat16),
            "sinS": sin_signed.astype(ml_dtypes.bfloat16),
            "mask_a": mask_a,
            "mask_b": mask_b,
        })
    return per_core


def _partial_to_full(partial):
    """Device partial [NCH, HIDDEN, TC] -> [HIDDEN, T] float64 (unscaled)."""
    p = np.asarray(partial, dtype=np.float64)
    out = p.transpose(1, 0, 2).reshape(HIDDEN, T)
    return out / (ATTN_SCALE * WO_SCALE)


# ------------------------- device program -------------------------

def build_program():
    nc = bacc.Bacc(None, target_bir_lowering=False)

    xb_d = nc.declare_dram_parameter("xb", [NCH, P, KH, TC], BF16, isOutput=False)
    wq_d = nc.declare_dram_parameter("wq", [HIDDEN + 1, QKV_ROWS], BF16, isOutput=False)
    wo_d = nc.declare_dram_parameter("wo", [KV_DIM, HIDDEN],
                                     F8E4 if FP8_OUT else BF16, isOutput=False)
    srow_d = nc.declare_dram_parameter("srow", [1, 2 * TC], F32, isOutput=False)
    cos_d = nc.declare_dram_parameter("cosT", [32, T], BF16, isOutput=False)
    sin_d = nc.declare_dram_parameter("sinS", [64, T], BF16, isOutput=False)
    maska_d = nc.declare_dram_parameter("mask_a", [P, TC], BF16, isOutput=False)
    maskb_d = nc.declare_dram_parameter("mask_b", [P, TC], BF16, isOutput=False)
    out_d = nc.declare_dram_parameter("partial", [NCH, HIDDEN, TC], BF16, isOutput=True)

    with tile.TileContext(nc) as tc:
        _body(tc, nc, xb_d, wq_d, wo_d, srow_d, cos_d, sin_d,
              maska_d, maskb_d, out_d)
    nc.compile()
    return nc


def _body(tc, nc, xb_d, wq_d, wo_d, srow_d, cos_d, sin_d, maska_d, maskb_d, out_d):
    ATTN_DT = F8E4 if FP8_OUT else BF16
    ctx = contextlib.ExitStack()
    with ctx:
        const = ctx.enter_context(tc.tile_pool(name="const", bufs=1))
        xbf = ctx.enter_context(tc.tile_pool(name="xbf", bufs=2))
        qkvp = ctx.enter_context(tc.tile_pool(name="qkvp", bufs=7))
        qsp = ctx.enter_context(tc.tile_pool(name="qsp", bufs=2))
        krp = ctx.enter_context(tc.tile_pool(name="krp", bufs=2))
        kprevp = ctx.enter_context(tc.tile_pool(name="kprevp", bufs=2))
        tmpp = ctx.enter_context(tc.tile_pool(name="tmpp", bufs=2))
        vnatp = ctx.enter_context(tc.tile_pool(name="vnatp", bufs=6))
        expp = ctx.enter_context(tc.tile_pool(name="expp", bufs=4))
        probp = ctx.enter_context(tc.tile_pool(name="probp", bufs=6))
        smallp = ctx.enter_context(tc.tile_pool(name="smallp", bufs=4))
        rsbp = ctx.enter_context(tc.tile_pool(name="rsbp", bufs=3))
        attnp = ctx.enter_context(tc.tile_pool(name="attnp", bufs=2))
        outsb = ctx.enter_context(tc.tile_pool(name="outsb", bufs=3))

        ps_sc = ctx.enter_context(tc.tile_pool(name="ps_sc", bufs=4, space="PSUM"))
        ps_av = ctx.enter_context(tc.tile_pool(name="ps_av", bufs=2, space="PSUM"))
        ps_op = ctx.enter_context(tc.tile_pool(name="ps_op", bufs=2, space="PSUM"))

        # ---- prologue DMAs ----
        # scalar queue: wq tiles then wo; sync queue: x chunk 0 pieces then
        # small consts.  The first qkv matmul needs wq tile 0 + x piece 0.
        wq_sb = const.tile([P, KH, QKV_ROWS], BF16)
        for k in range(KH):
            nc.scalar.dma_start(out=wq_sb[0:W_SIZES[k], k, :],
                                in_=wq_d[k * P:k * P + W_SIZES[k], :])
        wo_sb = const.tile([P, 4, HIDDEN], ATTN_DT)
        for kk in range(4):
            nc.scalar.dma_start(out=wo_sb[:, kk, :], in_=wo_d[kk * P:(kk + 1) * P, :])

        cos_sb = const.tile([P, T], BF16)
        sin_sb = const.tile([P, T], BF16)
        srow_sb = const.tile([1, Q_MULT, P], F32)
        mask_a = const.tile([P, TC], BF16)
        mask_b = const.tile([P, TC], BF16)

        def emit_small_dmas():
            for b in range(4):
                nc.sync.dma_start(out=cos_sb[32 * b:32 * (b + 1), :], in_=cos_d[:])
            for b in range(2):
                nc.sync.dma_start(out=sin_sb[64 * b:64 * (b + 1), :], in_=sin_d[:])
            nc.sync.dma_start(out=srow_sb[:], in_=srow_d[:])
            nc.sync.dma_start(out=mask_a[:], in_=maska_d[:])
            nc.sync.dma_start(out=mask_b[:], in_=maskb_d[:])

        # ---------------- emitters ----------------

        def make_qkv(ch):
            """Returns (dma_fn, [m_fn x5]); each m_fn emits 23 matmuls+drain."""
            state = {}

            def dma_fn():
                xall = xbf.tile([P, KH, TC], BF16, tag="xk", name=f"xall_{ch}")
                if ch == 0:
                    for k0, k1 in ((0, 6), (6, 12), (12, 18), (18, KH)):
                        nc.sync.dma_start(out=xall[:, k0:k1, :],
                                          in_=xb_d[ch, :, k0:k1, :])
                else:
                    nc.sync.dma_start(out=xall[:], in_=xb_d[ch])
                state["x"] = xall

            tiles = [None] * QKV_M

            def make_m(m):
                def m_fn():
                    xall = state["x"]
                    pq = ps_sc.tile([P, TC], F32, tag="sc", name=f"pq_{ch}_{m}")
                    for k in range(KH):
                        ws = W_SIZES[k]
                        nc.tensor.matmul(pq[:], wq_sb[0:ws, k, m * P:(m + 1) * P],
                                         xall[0:ws, k, :],
                                         start=(k == 0), stop=(k == KH - 1))
                    qm = qkvp.tile([P, TC], BF16, tag="qkv", name=f"qm_{ch}_{m}")
                    nc.scalar.activation(qm[:], pq[:], AF.Copy)
                    tiles[m] = qm
                return m_fn

            return dma_fn, [make_m(m) for m in range(QKV_M)], tiles

        def emit_rope(ch, qkv_tiles):
            """k tile first (m=4), then q tiles; returns (qs, kr)."""
            t0 = ch * TC
            qs = qsp.tile([64, Q_MULT, TC], BF16, tag="qs", name=f"qs_{ch}")
            kr = krp.tile([64, TC], BF16, tag="kr", name=f"kr_{ch}")
            for m in (4, 0, 1, 2, 3):
                rows = P if m < 4 else HEAD_DIM
                src = qkv_tiles[m]
                sw = tmpp.tile([P, TC], BF16, tag="sw", bufs=1, name=f"sw_{ch}_{m}")
                for b in range(rows // 64):
                    nc.vector.tensor_copy(sw[64 * b:64 * b + 32, :],
                                          src[64 * b + 32:64 * b + 64, :])
                    nc.vector.tensor_copy(sw[64 * b + 32:64 * b + 64, :],
                                          src[64 * b:64 * b + 32, :])
                t1 = tmpp.tile([P, TC], BF16, tag="t1", bufs=1, name=f"t1_{ch}_{m}")
                nc.vector.tensor_mul(t1[0:rows, :], src[0:rows, :],
                                     cos_sb[0:rows, t0:t0 + TC])
                t2 = tmpp.tile([P, TC], BF16, tag="t2", bufs=1, name=f"t2_{ch}_{m}")
                nc.vector.tensor_mul(t2[0:rows, :], sw[0:rows, :],
                                     sin_sb[0:rows, t0:t0 + TC])
                if m < 4:
                    qr = tmpp.tile([P, TC], BF16, tag="qr", bufs=1, name=f"qr_{ch}_{m}")
                    nc.vector.tensor_add(qr[:], t1[:], t2[:])
                    nc.vector.tensor_copy(qs[:, 2 * m, :], qr[0:64, :])
                    nc.vector.tensor_copy(qs[:, 2 * m + 1, :], qr[64:P, :])
                else:
                    nc.vector.tensor_add(kr[:], t1[0:64, :], t2[0:64, :])
            return qs, kr

        def emit_vnat(ch, qkv_tiles):
            vn_tiles = []
            for tau in range(NT):
                vn = vnatp.tile([P, P], BF16, tag="vn", name=f"vn_{ch}_{tau}")
                nc.vector.memset(vn[:, HEAD_DIM:P], 0.0)
                nc.vector.memset(vn[:, HEAD_DIM:HEAD_DIM + 1], 1.0)
                nc.sync.dma_start(
                    out=vn[:, 0:HEAD_DIM],
                    in_=qkv_tiles[4][HEAD_DIM:P, tau * P:(tau + 1) * P],
                    transpose=True)
                vn_tiles.append(vn)
            return vn_tiles

        def emit_att_tau(ch, tau, qs, kr, vn_tiles, attn_t, kprev, vnat_prev):
            tg = ch * NT + tau
            kprev_blk = (kr[:, (tau - 1) * P:tau * P] if tau > 0
                         else (kprev[:, :] if kprev is not None else None))
            vprev_blk = vn_tiles[tau - 1] if tau > 0 else vnat_prev
            for hg in range(2):
                qblk = qs[:, hg * 4:(hg + 1) * 4, tau * P:(tau + 1) * P]
                psc_a = ps_sc.tile([P, TC], F32, tag="sc", name=f"pa_{hg}")
                nc.tensor.matmul(psc_a[:], kr[:, tau * P:(tau + 1) * P], qblk,
                                 start=True, stop=True)
                et_a = expp.tile([P, TC], BF16, tag="et", name=f"ea_{hg}")
                nc.scalar.activation(et_a[:], psc_a[:], AF.Exp)
                pr_a = probp.tile([P, TC], BF16, tag="pr", name=f"fa_{hg}")
                eng_a = nc.vector if hg == 0 else nc.gpsimd
                eng_a.tensor_mul(pr_a[:], et_a[:], mask_a[:])
                pr_b = None
                if tg > 0:
                    psc_b = ps_sc.tile([P, TC], F32, tag="sc", name=f"pb_{hg}")
                    nc.tensor.matmul(psc_b[:], kprev_blk, qblk,
                                     start=True, stop=True)
                    et_b = expp.tile([P, TC], BF16, tag="et", name=f"eb_{hg}")
                    nc.scalar.activation(et_b[:], psc_b[:], AF.Exp)
                    pr_b = probp.tile([P, TC], BF16, tag="pr", name=f"fb_{hg}")
                    eng_b = nc.gpsimd if hg == 0 else nc.vector
                    eng_b.tensor_mul(pr_b[:], et_b[:], mask_b[:])

                pav = ps_av.tile([P, NT, P], F32, tag="av", name=f"pv_{hg}")
                nc.tensor.matmul(pav[:], vn_tiles[tau][:], pr_a[:],
                                 start=True, stop=(tg == 0))
                if tg > 0:
                    nc.tensor.matmul(pav[:], vprev_blk[:], pr_b[:],
                                     start=False, stop=True)

                den = smallp.tile([1, NT, P], F32, tag="den", name=f"dn_{hg}")
                nc.vector.scalar_tensor_tensor(
                    den[:], pav[64:65, :, :], 1.0 / ATTN_SCALE,
                    srow_sb[0:1, hg * 4:(hg + 1) * 4, :],
                    op0=ALU.mult, op1=ALU.add)
                rr = smallp.tile([1, NT, P], F32, tag="rr", name=f"rr_{hg}")
                nc.vector.reciprocal_approx_fast(rr[:], den[:])
                rsb = rsbp.tile([HEAD_DIM, NT, P], F32, tag="rsb", name=f"rs_{hg}")
                nc.gpsimd.partition_broadcast(rsb[:], rr[:])
                nc.vector.tensor_mul(
                    attn_t[hg * HEAD_DIM:(hg + 1) * HEAD_DIM, :, tau * P:(tau + 1) * P],
                    pav[0:HEAD_DIM, :, :], rsb[:])

        def make_op(ch, attn_t):
            """23 out-proj c-tile emitters for chunk ch."""
            def make_c(c, ci):
                cs = C_SIZES[c]

                def c_fn():
                    po = ps_op.tile([P, TC], F32, tag="op", name=f"po_{ch}_{c}")
                    if FP8_OUT:
                        for k2 in range(2):
                            nc.tensor.matmul(
                                po[0:cs, :],
                                wo_sb[:, 2 * k2:2 * k2 + 2, c * P:c * P + cs],
                                attn_t[:, 2 * k2:2 * k2 + 2, :],
                                start=(k2 == 0), stop=(k2 == 1),
                                perf_mode=mybir.MatmulPerfMode.DoubleRow)
                    else:
                        for kk in range(4):
                            nc.tensor.matmul(
                                po[0:cs, :], wo_sb[:, kk, c * P:c * P + cs],
                                attn_t[:, kk, :],
                                start=(kk == 0), stop=(kk == 3))
                    ot = outsb.tile([P, TC], BF16, tag="ot", name=f"ot_{ch}_{c}")
                    # gpsimd cannot read PSUM; split drains scalar/vector
                    if ci % 3 != 2:
                        nc.scalar.activation(ot[0:cs, :], po[0:cs, :], AF.Copy)
                    else:
                        nc.vector.tensor_copy(ot[0:cs, :], po[0:cs, :])
                    nc.scalar.dma_start(out=out_d[ch, c * P:c * P + cs, :],
                                        in_=ot[0:cs, :])
                return c_fn

            return [make_c(c, ci) for ci, c in enumerate(range(CD))]

        # ---------------- schedule ----------------
        dma0, qkv_m_fns, qkv_tiles_cur = make_qkv(0)
        dma0()
        emit_small_dmas()
        for fn in qkv_m_fns:
            fn()

        kprev = None
        vnat_prev = None
        op_prev = []       # out-proj emitters for chunk ch-1
        attn_prev = None

        for ch in range(NCH):
            qs, kr = emit_rope(ch, qkv_tiles_cur)
            vn_tiles = emit_vnat(ch, qkv_tiles_cur)
            attn_t = attnp.tile([P, NT, TC], ATTN_DT, tag="attn", name=f"attn_{ch}")

            if ch + 1 < NCH:
                dma_n, qkv_n_fns, qkv_tiles_next = make_qkv(ch + 1)
            else:
                dma_n, qkv_n_fns, qkv_tiles_next = None, [], None

            # op split 6/6/6/5 across the 4 taus
            op_split = [op_prev[0:6], op_prev[6:12], op_prev[12:18], op_prev[18:23]]

            for tau in range(NT):
                for fn in op_split[tau]:
                    fn()
                emit_att_tau(ch, tau, qs, kr, vn_tiles, attn_t, kprev, vnat_prev)
                if dma_n is not None and tau == 0:
                    dma_n()
                if tau < len(qkv_n_fns):
                    qkv_n_fns[tau]()
                if tau == NT - 1 and len(qkv_n_fns) > 4:
                    qkv_n_fns[4]()

            kprev_t = kprevp.tile([HEAD_DIM, P], BF16, tag="kp", name=f"kp_{ch}")
            nc.vector.tensor_copy(kprev_t[:], kr[:, TC - P:TC])
            kprev = kprev_t
            vnat_prev = vn_tiles[NT - 1]
            op_prev = make_op(ch, attn_t)
            attn_prev = attn_t
            qkv_tiles_cur = qkv_tiles_next

        # epilogue: out-proj of the last chunk
        for fn in op_prev:
            fn()


# ------------------------- entry point -------------------------

def _get_program():
    if "nc" not in _CACHE:
        _CACHE["nc"] = build_program()
    return _CACHE["nc"]


def run_cores(inputs, trace=False):
    per_core = _host_prepare(**inputs)
    nc = _get_program()
    res = bass_utils.run_bass_kernel_spmd(
        nc, per_core, core_ids=list(range(N_CORES)), trace=trace,
    )
    return res


def kernel(**inputs):
    res = run_cores(inputs)
    acc = np.zeros((HIDDEN, T), dtype=np.float64)
    for r in res.results:
        acc += _partial_to_full(r["partial"])
    out = acc.T + inputs["x"][0].astype(np.float64) + inputs["b_out"][None, :].astype(np.float64)
    return out[None].astype(np.float32)


# revision 5
# speedup vs baseline: 1.6981x; 1.6981x over previous
"""Trainium2 Bass kernel for nn_AttentionBlock (sliding-window attention block).

Distribution: tensor-parallel over the 8 KV head groups (one group per core).
Each core computes qkv^T for its group (640 rows x 2048 tokens), windowed
attention for its 8 q-heads, and a partial output projection; host sums the
8 partials and adds x + b_out.

Device layout is feature-major: features on partitions, tokens on the free
dim.  Host pre-transposes and pre-rmsnorms x (bf16) and the weights.

Key structure (v3):
- x shipped pre-normalized (x*sqrt(H)/s); bias enters via a ones u-row.
- xb chunk-contiguous in DRAM ([NCH, P, KH, TC]), loaded in 4 pieces per
  chunk so the qkv k-loop can start before the full chunk lands.
- wq shipped partition-major ([P, M, KH, 128]) so each m-slice loads with
  one fat DMA; k-tiles zero-padded to 128 rows.
- softmax denominator: sink added on DVE (scalar_tensor_tensor), reciprocal
  on DVE, partition-broadcast on GpSimd -- no PE rank-1 matmuls.
- attention output tile is [128, 4, TC] with head h -> (kk=h%4, half=h//4)
  so the normalize multiply writes it directly (no layout copies); w_out is
  host-reordered to match.
- per-tau PE queue: [scores][op 3c][qkv m][pav][op 3c] so the PE never
  waits on the exp/mask/normalize chains; out-proj of chunk ch-1 and qkv of
  ch+1 fill all gaps.  GpSimd runs only partition_broadcast + rope copies
  (it is slow and must stay off the critical path).
- optional fp8 out-projection (DoubleRow) controlled by FP8_OUT.
"""

import contextlib
import math

import numpy as np
import ml_dtypes

import concourse.bass as bass
import concourse.mybir as mybir
import concourse.tile as tile
from concourse import bacc, bass_utils

# ---- problem config (hardcoded from the reference) ----
HIDDEN = 2880
HEAD_DIM = 64
N_HEADS = 64
N_KV = 8
Q_MULT = N_HEADS // N_KV  # 8
SLIDING_WINDOW = 128
ROPE_BASE = 150000.0
ROPE_SCALE = 32.0
NTK_ALPHA = 1.0
NTK_BETA = 32.0
INIT_CTX = 4096
RMS_EPS = 1e-5
SM_SCALE = 1.0 / math.sqrt(HEAD_DIM)
Q_DIM = N_HEADS * HEAD_DIM  # 4096
KV_DIM = N_KV * HEAD_DIM  # 512
B, T = 1, 2048

N_CORES = 8
P = 128
TC = 512  # token chunk
NCH = T // TC  # 4 chunks
NT = TC // P  # 4 token blocks per chunk
KH = 23  # hidden tiles: 23x128 (last zero-padded, carries the bias u-row)
KHP = KH * P  # 2944
QKV_ROWS = Q_MULT * HEAD_DIM + 2 * HEAD_DIM  # 640
QKV_M = QKV_ROWS // P  # 5
CD = 23  # out-proj c tiles: 22x128 + 1x64
C_SIZES = [128] * 22 + [64]

F32 = mybir.dt.float32
BF16 = mybir.dt.bfloat16
F8E4 = mybir.dt.float8e4
AF = mybir.ActivationFunctionType
ALU = mybir.AluOpType

FP8_OUT = False           # fp8 (DoubleRow) out-projection
ATTN_SCALE = 8.0 if FP8_OUT else 1.0
WO_SCALE = 64.0 if FP8_OUT else 1.0

_CACHE = {}


# ------------------------- host-side preparation -------------------------

def _rope_tables():
    d_half = HEAD_DIM // 2
    freq = ROPE_BASE ** (np.arange(0, HEAD_DIM, 2, dtype=np.float64) / HEAD_DIM)
    concentration = 0.1 * math.log(ROPE_SCALE) + 1.0
    low = d_half * math.log(INIT_CTX / (NTK_BETA * 2 * math.pi)) / math.log(ROPE_BASE)
    high = d_half * math.log(INIT_CTX / (NTK_ALPHA * 2 * math.pi)) / math.log(ROPE_BASE)
    interpolation = 1.0 / (ROPE_SCALE * freq)
    extrapolation = 1.0 / freq
    ramp = (np.arange(d_half, dtype=np.float64) - low) / (high - low)
    mask = 1.0 - np.clip(ramp, 0.0, 1.0)
    inv_freq = interpolation * (1.0 - mask) + extrapolation * mask
    pos = np.arange(T, dtype=np.float64)
    angles = pos[:, None] * inv_freq[None, :]  # [T, 32]
    cos = (np.cos(angles) * concentration).astype(np.float32)
    sin = (np.sin(angles) * concentration).astype(np.float32)
    return cos.T.copy(), sin.T.copy()  # [32, T]


def _perm64():
    # evens then odds within a 64-dim head
    return np.concatenate([np.arange(0, 64, 2), np.arange(1, 64, 2)])


def _host_prepare(x, norm_scale, w_qkv, b_qkv, sinks, w_out, b_out):
    x64 = x[0].astype(np.float64)
    s = np.sqrt((x64 * x64).sum(axis=1) + HIDDEN * RMS_EPS)  # [2048]
    xn = x64 * (math.sqrt(HIDDEN) / s)[:, None]  # pre-rmsnormed, ~N(0,1)
    xf = np.zeros((KHP, T), dtype=np.float32)
    xf[:HIDDEN] = xn.T
    xf[HIDDEN] = 1.0  # u row (bias)
    # chunk-contiguous: [NCH, P, KH, TC]
    xb = np.ascontiguousarray(
        xf.reshape(KH, P, NCH, TC).transpose(2, 1, 0, 3)).astype(ml_dtypes.bfloat16)

    w_eff = (w_qkv * norm_scale[None, :]).astype(np.float64)
    b_eff = b_qkv.astype(np.float64).copy()
    # fold softmax scale into q rows (rope is a rotation; scale commutes)
    w_eff[:Q_DIM] *= SM_SCALE
    b_eff[:Q_DIM] *= SM_SCALE

    perm = _perm64()
    cosT, sinT = _rope_tables()
    sin_signed = np.concatenate([-sinT, sinT], axis=0)  # [64, T]

    # band masks for the two score blocks, repeated over 4 heads
    pidx = np.arange(P)[:, None]
    jidx = np.arange(P)[None, :]
    mask_a = np.tile((jidx >= pidx), (1, 4)).astype(ml_dtypes.bfloat16)  # [128, 512]
    mask_b = np.tile((jidx <= pidx), (1, 4)).astype(ml_dtypes.bfloat16)

    per_core = []
    for g in range(N_CORES):
        rows = []
        for h in range(Q_MULT):  # q heads of this group, rope-permuted
            base = (g * Q_MULT + h) * HEAD_DIM
            rows.append(base + perm)
        rows.append(Q_DIM + g * HEAD_DIM + perm)  # k head, rope-permuted
        rows.append(Q_DIM + KV_DIM + g * HEAD_DIM + np.arange(HEAD_DIM))  # v natural
        rows = np.concatenate(rows)
        wq_g = np.zeros((KHP, QKV_ROWS), dtype=np.float64)
        wq_g[:HIDDEN] = w_eff[rows].T
        wq_g[HIDDEN] = b_eff[rows]
        # partition-major for fat per-m DMAs: [P, M, KH, 128]
        wq_g = np.ascontiguousarray(
            wq_g.reshape(KH, P, QKV_M, P).transpose(1, 2, 0, 3)
        ).astype(ml_dtypes.bfloat16)

        # w_out columns for this group, reordered so attn row r of kk-tile j is:
        #   rows 0:64  -> head j      (hg0)
        #   rows 64:128-> head 4 + j  (hg1)
        wo_local = w_out[:, g * KV_DIM:(g + 1) * KV_DIM].T  # [512, 2880]
        order = []
        for kk in range(4):
            order.extend(range(kk * 64, kk * 64 + 64))
            order.extend(range((4 + kk) * 64, (4 + kk) * 64 + 64))
        wo_g = np.ascontiguousarray(wo_local[order])  # [512, 2880]
        if FP8_OUT:
            wo_g = np.clip(wo_g * WO_SCALE, -240, 240).astype(ml_dtypes.float8_e4m3)
        else:
            wo_g = wo_g.astype(ml_dtypes.bfloat16)

        sexp = np.exp2(sinks[g * Q_MULT:(g + 1) * Q_MULT]).astype(np.float64)
        srow = np.repeat(sexp / ATTN_SCALE, P).reshape(1, -1).astype(np.float32)
        per_core.append({
            "xb": xb,
            "wq": wq_g,
            "wo": wo_g,
            "srow": srow,  # [1, 1024] f32, sink/ATTN_SCALE per head
            "cosT": cosT.astype(ml_dtypes.bfloat16),
            "sinS": sin_signed.astype(ml_dtypes.bfloat16),
            "mask_a": mask_a,
            "mask_b": mask_b,
        })
    return per_core


def _partial_to_full(partial):
    """Device partial [NCH, HIDDEN, TC] -> [HIDDEN, T] float64 (unscaled)."""
    p = np.asarray(partial, dtype=np.float64)
    out = p.transpose(1, 0, 2).reshape(HIDDEN, T)
    return out / (ATTN_SCALE * WO_SCALE)


# ------------------------- device program -------------------------

def build_program():
    nc = bacc.Bacc(None, target_bir_lowering=False)

    xb_d = nc.declare_dram_parameter("xb", [NCH, P, KH, TC], BF16, isOutput=False)
    wq_d = nc.declare_dram_parameter("wq", [P, QKV_M, KH, P], BF16, isOutput=False)
    wo_d = nc.declare_dram_parameter("wo", [KV_DIM, HIDDEN],
                                     F8E4 if FP8_OUT else BF16, isOutput=False)
    srow_d = nc.declare_dram_parameter("srow", [1, 2 * TC], F32, isOutput=False)
    cos_d = nc.declare_dram_parameter("cosT", [32, T], BF16, isOutput=False)
    sin_d = nc.declare_dram_parameter("sinS", [64, T], BF16, isOutput=False)
    maska_d = nc.declare_dram_parameter("mask_a", [P, TC], BF16, isOutput=False)
    maskb_d = nc.declare_dram_parameter("mask_b", [P, TC], BF16, isOutput=False)
    out_d = nc.declare_dram_parameter("partial", [NCH, HIDDEN, TC], BF16, isOutput=True)

    with tile.TileContext(nc) as tc:
        _body(tc, nc, xb_d, wq_d, wo_d, srow_d, cos_d, sin_d,
              maska_d, maskb_d, out_d)
    nc.compile()
    return nc


def _body(tc, nc, xb_d, wq_d, wo_d, srow_d, cos_d, sin_d, maska_d, maskb_d, out_d):
    ATTN_DT = F8E4 if FP8_OUT else BF16
    ctx = contextlib.ExitStack()
    with ctx:
        const = ctx.enter_context(tc.tile_pool(name="const", bufs=1))
        xbf = ctx.enter_context(tc.tile_pool(name="xbf", bufs=2))
        qkvp = ctx.enter_context(tc.tile_pool(name="qkvp", bufs=7))
        qsp = ctx.enter_context(tc.tile_pool(name="qsp", bufs=2))
        krp = ctx.enter_context(tc.tile_pool(name="krp", bufs=2))
        kprevp = ctx.enter_context(tc.tile_pool(name="kprevp", bufs=2))
        tmpp = ctx.enter_context(tc.tile_pool(name="tmpp", bufs=2))
        vnatp = ctx.enter_context(tc.tile_pool(name="vnatp", bufs=6))
        expp = ctx.enter_context(tc.tile_pool(name="expp", bufs=4))
        probp = ctx.enter_context(tc.tile_pool(name="probp", bufs=6))
        smallp = ctx.enter_context(tc.tile_pool(name="smallp", bufs=3))
        rsbp = ctx.enter_context(tc.tile_pool(name="rsbp", bufs=3))
        attnp = ctx.enter_context(tc.tile_pool(name="attnp", bufs=2))
        outsb = ctx.enter_context(tc.tile_pool(name="outsb", bufs=3))

        ps_sc = ctx.enter_context(tc.tile_pool(name="ps_sc", bufs=4, space="PSUM"))
        ps_av = ctx.enter_context(tc.tile_pool(name="ps_av", bufs=2, space="PSUM"))
        ps_op = ctx.enter_context(tc.tile_pool(name="ps_op", bufs=2, space="PSUM"))

        # ---- prologue DMAs ----
        # scalar queue: wq m-slices then wo; sync queue: x chunk 0 pieces then
        # small consts.  First qkv matmul needs wq m-slice 0 + x piece 0.
        wq_sb = const.tile([P, QKV_M, KH, P], BF16)
        for m in range(QKV_M):
            nc.scalar.dma_start(out=wq_sb[:, m], in_=wq_d[:, m])
        wo_sb = const.tile([P, 4, HIDDEN], ATTN_DT)
        for kk in range(4):
            nc.scalar.dma_start(out=wo_sb[:, kk, :], in_=wo_d[kk * P:(kk + 1) * P, :])

        cos_sb = const.tile([P, T], BF16)
        sin_sb = const.tile([P, T], BF16)
        srow_sb = const.tile([1, Q_MULT, P], F32)
        mask_a = const.tile([P, TC], BF16)
        mask_b = const.tile([P, TC], BF16)

        def emit_small_dmas():
            for b in range(4):
                nc.sync.dma_start(out=cos_sb[32 * b:32 * (b + 1), :], in_=cos_d[:])
            for b in range(2):
                nc.sync.dma_start(out=sin_sb[64 * b:64 * (b + 1), :], in_=sin_d[:])
            nc.sync.dma_start(out=srow_sb[:], in_=srow_d[:])
            nc.sync.dma_start(out=mask_a[:], in_=maska_d[:])
            nc.sync.dma_start(out=mask_b[:], in_=maskb_d[:])

        # ---------------- emitters ----------------

        def make_qkv(ch):
            """Returns (dma_fn, [m_fn x5], tiles); m_fn emits 23 matmuls+drain."""
            state = {}

            def dma_fn():
                xall = xbf.tile([P, KH, TC], BF16, tag="xk", name=f"xall_{ch}")
                for k0, k1 in ((0, 6), (6, 12), (12, 18), (18, KH)):
                    nc.sync.dma_start(out=xall[:, k0:k1, :],
                                      in_=xb_d[ch, :, k0:k1, :])
                state["x"] = xall

            tiles = [None] * QKV_M

            def make_m(m):
                def m_fn():
                    xall = state["x"]
                    pq = ps_sc.tile([P, TC], F32, tag="sc", name=f"pq_{ch}_{m}")
                    for k in range(KH):
                        nc.tensor.matmul(pq[:], wq_sb[:, m, k, :],
                                         xall[:, k, :],
                                         start=(k == 0), stop=(k == KH - 1))
                    qm = qkvp.tile([P, TC], BF16, tag="qkv", name=f"qm_{ch}_{m}")
                    nc.scalar.activation(qm[:], pq[:], AF.Copy)
                    tiles[m] = qm
                return m_fn

            return dma_fn, [make_m(m) for m in range(QKV_M)], tiles

        def emit_rope(ch, qkv_tiles):
            """k tile first (m=4), then q tiles; returns (qs, kr)."""
            t0 = ch * TC
            qs = qsp.tile([64, Q_MULT, TC], BF16, tag="qs", name=f"qs_{ch}")
            kr = krp.tile([64, TC], BF16, tag="kr", name=f"kr_{ch}")
            for m in (4, 0, 1, 2, 3):
                rows = P if m < 4 else HEAD_DIM
                src = qkv_tiles[m]
                sw = tmpp.tile([P, TC], BF16, tag="sw", bufs=1, name=f"sw_{ch}_{m}")
                for b in range(rows // 64):
                    nc.vector.tensor_copy(sw[64 * b:64 * b + 32, :],
                                          src[64 * b + 32:64 * b + 64, :])
                    nc.vector.tensor_copy(sw[64 * b + 32:64 * b + 64, :],
                                          src[64 * b:64 * b + 32, :])
                t1 = tmpp.tile([P, TC], BF16, tag="t1", bufs=1, name=f"t1_{ch}_{m}")
                nc.vector.tensor_mul(t1[0:rows, :], src[0:rows, :],
                                     cos_sb[0:rows, t0:t0 + TC])
                t2 = tmpp.tile([P, TC], BF16, tag="t2", bufs=1, name=f"t2_{ch}_{m}")
                nc.vector.tensor_mul(t2[0:rows, :], sw[0:rows, :],
                                     sin_sb[0:rows, t0:t0 + TC])
                if m < 4:
                    qr = tmpp.tile([P, TC], BF16, tag="qr", bufs=1, name=f"qr_{ch}_{m}")
                    nc.vector.tensor_add(qr[:], t1[:], t2[:])
                    nc.gpsimd.tensor_copy(qs[:, 2 * m, :], qr[0:64, :])
                    nc.gpsimd.tensor_copy(qs[:, 2 * m + 1, :], qr[64:P, :])
                else:
                    nc.vector.tensor_add(kr[:], t1[0:64, :], t2[0:64, :])
            return qs, kr

        def emit_vnat(ch, qkv_tiles):
            vn_tiles = []
            for tau in range(NT):
                vn = vnatp.tile([P, P], BF16, tag="vn", name=f"vn_{ch}_{tau}")
                nc.vector.memset(vn[:, HEAD_DIM:P], 0.0)
                nc.vector.memset(vn[:, HEAD_DIM:HEAD_DIM + 1], 1.0)
                nc.sync.dma_start(
                    out=vn[:, 0:HEAD_DIM],
                    in_=qkv_tiles[4][HEAD_DIM:P, tau * P:(tau + 1) * P],
                    transpose=True)
                vn_tiles.append(vn)
            return vn_tiles

        def emit_scores_tau(ch, tau, qs, kr, kprev):
            """Score matmuls + exps + mask-muls for both head groups."""
            tg = ch * NT + tau
            kprev_blk = (kr[:, (tau - 1) * P:tau * P] if tau > 0
                         else (kprev[:, :] if kprev is not None else None))
            prs = []
            ets = []
            for hg in range(2):
                qblk = qs[:, hg * 4:(hg + 1) * 4, tau * P:(tau + 1) * P]
                psc_a = ps_sc.tile([P, TC], F32, tag="sc", name=f"pa_{hg}")
                nc.tensor.matmul(psc_a[:], kr[:, tau * P:(tau + 1) * P], qblk,
                                 start=True, stop=True)
                et_a = expp.tile([P, TC], BF16, tag="et", name=f"ea_{hg}")
                nc.scalar.activation(et_a[:], psc_a[:], AF.Exp)
                psc_b = et_b = None
                if tg > 0:
                    psc_b = ps_sc.tile([P, TC], F32, tag="sc", name=f"pb_{hg}")
                    nc.tensor.matmul(psc_b[:], kprev_blk, qblk,
                                     start=True, stop=True)
                    et_b = expp.tile([P, TC], BF16, tag="et", name=f"eb_{hg}")
                    nc.scalar.activation(et_b[:], psc_b[:], AF.Exp)
                ets.append((et_a, et_b))
            for hg in range(2):
                et_a, et_b = ets[hg]
                pr_a = probp.tile([P, TC], BF16, tag="pr", name=f"fa_{hg}")
                nc.vector.tensor_mul(pr_a[:], et_a[:], mask_a[:])
                pr_b = None
                if et_b is not None:
                    pr_b = probp.tile([P, TC], BF16, tag="pr", name=f"fb_{hg}")
                    nc.vector.tensor_mul(pr_b[:], et_b[:], mask_b[:])
                prs.append((pr_a, pr_b))
            return prs

        def emit_pav_tau(ch, tau, prs, vn_tiles, attn_t, vnat_prev):
            """prob@v matmuls + normalize chain for both head groups."""
            tg = ch * NT + tau
            vprev_blk = vn_tiles[tau - 1] if tau > 0 else vnat_prev
            pavs = []
            for hg in range(2):
                pr_a, pr_b = prs[hg]
                pav = ps_av.tile([P, NT, P], F32, tag="av", name=f"pv_{hg}")
                nc.tensor.matmul(pav[:], vn_tiles[tau][:], pr_a[:],
                                 start=True, stop=(tg == 0))
                if tg > 0:
                    nc.tensor.matmul(pav[:], vprev_blk[:], pr_b[:],
                                     start=False, stop=True)
                pavs.append(pav)
            rrs = []
            for hg in range(2):
                den = smallp.tile([1, NT, P], F32, tag="den", name=f"dn_{hg}")
                nc.vector.scalar_tensor_tensor(
                    den[:], pavs[hg][64:65, :, :], 1.0 / ATTN_SCALE,
                    srow_sb[0:1, hg * 4:(hg + 1) * 4, :],
                    op0=ALU.mult, op1=ALU.add)
                rr = smallp.tile([1, NT, P], F32, tag="rr", name=f"rr_{hg}")
                nc.vector.reciprocal_approx_fast(rr[:], den[:])
                rrs.append(rr)
            rsbs = []
            for hg in range(2):
                rsb = rsbp.tile([HEAD_DIM, NT, P], F32, tag="rsb", name=f"rs_{hg}")
                nc.gpsimd.partition_broadcast(rsb[:], rrs[hg][:])
                rsbs.append(rsb)
            for hg in range(2):
                nc.vector.tensor_mul(
                    attn_t[hg * HEAD_DIM:(hg + 1) * HEAD_DIM, :, tau * P:(tau + 1) * P],
                    pavs[hg][0:HEAD_DIM, :, :], rsbs[hg][:])

        def make_op(ch, attn_t):
            """23 out-proj c-tile emitters for chunk ch."""
            def make_c(c, ci):
                cs = C_SIZES[c]

                def c_fn():
                    po = ps_op.tile([P, TC], F32, tag="op", name=f"po_{ch}_{c}")
                    if FP8_OUT:
                        for k2 in range(2):
                            nc.tensor.matmul(
                                po[0:cs, :],
                                wo_sb[:, 2 * k2:2 * k2 + 2, c * P:c * P + cs],
                                attn_t[:, 2 * k2:2 * k2 + 2, :],
                                start=(k2 == 0), stop=(k2 == 1),
                                perf_mode=mybir.MatmulPerfMode.DoubleRow)
                    else:
                        for kk in range(4):
                            nc.tensor.matmul(
                                po[0:cs, :], wo_sb[:, kk, c * P:c * P + cs],
                                attn_t[:, kk, :],
                                start=(kk == 0), stop=(kk == 3))
                    ot = outsb.tile([P, TC], BF16, tag="ot", name=f"ot_{ch}_{c}")
                    # gpsimd cannot read PSUM; split drains scalar/vector
                    if ci % 3 != 2:
                        nc.scalar.activation(ot[0:cs, :], po[0:cs, :], AF.Copy)
                    else:
                        nc.vector.tensor_copy(ot[0:cs, :], po[0:cs, :])
                    nc.sync.dma_start(out=out_d[ch, c * P:c * P + cs, :],
                                      in_=ot[0:cs, :])
                return c_fn

            return [make_c(c, ci) for ci, c in enumerate(range(CD))]

        # ---------------- schedule ----------------
        dma0, qkv_m_fns, qkv_tiles_cur = make_qkv(0)
        dma0()
        emit_small_dmas()
        for fn in qkv_m_fns:
            fn()

        kprev = None
        vnat_prev = None
        op_prev = []       # out-proj emitters for chunk ch-1

        for ch in range(NCH):
            qs, kr = emit_rope(ch, qkv_tiles_cur)
            vn_tiles = emit_vnat(ch, qkv_tiles_cur)
            attn_t = attnp.tile([P, NT, TC], ATTN_DT, tag="attn", name=f"attn_{ch}")

            if ch + 1 < NCH:
                dma_n, qkv_n_fns, qkv_tiles_next = make_qkv(ch + 1)
                dma_n()  # start the next x load before out-DMAs queue up
            else:
                qkv_n_fns, qkv_tiles_next = [], None

            # out-proj of ch-1 split 3+3 around each tau's work
            opq = list(op_prev)

            def pop_ops(n):
                for fn in opq[:n]:
                    fn()
                del opq[:n]

            for tau in range(NT):
                prs = emit_scores_tau(ch, tau, qs, kr, kprev)
                pop_ops(3)
                if tau < len(qkv_n_fns):
                    qkv_n_fns[tau]()
                emit_pav_tau(ch, tau, prs, vn_tiles, attn_t, vnat_prev)
                pop_ops(3 if tau < NT - 1 else 5)
                if tau == NT - 1 and len(qkv_n_fns) > 4:
                    qkv_n_fns[4]()

            kprev_t = kprevp.tile([HEAD_DIM, P], BF16, tag="kp", name=f"kp_{ch}")
            nc.vector.tensor_copy(kprev_t[:], kr[:, TC - P:TC])
            kprev = kprev_t
            vnat_prev = vn_tiles[NT - 1]
            op_prev = make_op(ch, attn_t)
            qkv_tiles_cur = qkv_tiles_next

        # epilogue: out-proj of the last chunk
        for fn in op_prev:
            fn()


# ------------------------- entry point -------------------------

def _get_program():
    if "nc" not in _CACHE:
        _CACHE["nc"] = build_program()
    return _CACHE["nc"]


def run_cores(inputs, trace=False):
    per_core = _host_prepare(**inputs)
    nc = _get_program()
    res = bass_utils.run_bass_kernel_spmd(
        nc, per_core, core_ids=list(range(N_CORES)), trace=trace,
    )
    return res


def kernel(**inputs):
    res = run_cores(inputs)
    acc = np.zeros((HIDDEN, T), dtype=np.float64)
    for r in res.results:
        acc += _partial_to_full(r["partial"])
    out = acc.T + inputs["x"][0].astype(np.float64) + inputs["b_out"][None, :].astype(np.float64)
    return out[None].astype(np.float32)


# revision 9
# speedup vs baseline: 2.0488x; 1.2066x over previous
"""Trainium2 Bass kernel for nn_AttentionBlock (sliding-window attention block).

Distribution: tensor-parallel over the 8 KV head groups (one group per core).
Each core computes qkv^T for its group (640 rows x 2048 tokens), windowed
attention for its 8 q-heads, and a partial output projection; host sums the
8 partials and adds x + b_out.

Device layout is feature-major: features on partitions, tokens on the free
dim.  Host pre-transposes and pre-rmsnorms x (bf16) and the weights.

Key structure (v3):
- x shipped pre-normalized (x*sqrt(H)/s); bias enters via a ones u-row.
- xb chunk-contiguous in DRAM ([NCH, P, KH, TC]), loaded in 4 pieces per
  chunk so the qkv k-loop can start before the full chunk lands.
- wq shipped partition-major ([P, M, KH, 128]) so each m-slice loads with
  one fat DMA; k-tiles zero-padded to 128 rows.
- softmax denominator: sink added on DVE (scalar_tensor_tensor), reciprocal
  on DVE, partition-broadcast on GpSimd -- no PE rank-1 matmuls.
- attention output tile is [128, 4, TC] with head h -> (kk=h%4, half=h//4)
  so the normalize multiply writes it directly (no layout copies); w_out is
  host-reordered to match.
- per-tau PE queue: [scores][op 3c][qkv m][pav][op 3c] so the PE never
  waits on the exp/mask/normalize chains; out-proj of chunk ch-1 and qkv of
  ch+1 fill all gaps.  GpSimd runs only partition_broadcast + rope copies
  (it is slow and must stay off the critical path).
- optional fp8 out-projection (DoubleRow) controlled by FP8_OUT.
"""

import contextlib
import math

import numpy as np
import ml_dtypes

import concourse.bass as bass
import concourse.mybir as mybir
import concourse.tile as tile
from concourse import bacc, bass_utils

# ---- problem config (hardcoded from the reference) ----
HIDDEN = 2880
HEAD_DIM = 64
N_HEADS = 64
N_KV = 8
Q_MULT = N_HEADS // N_KV  # 8
SLIDING_WINDOW = 128
ROPE_BASE = 150000.0
ROPE_SCALE = 32.0
NTK_ALPHA = 1.0
NTK_BETA = 32.0
INIT_CTX = 4096
RMS_EPS = 1e-5
SM_SCALE = 1.0 / math.sqrt(HEAD_DIM)
Q_DIM = N_HEADS * HEAD_DIM  # 4096
KV_DIM = N_KV * HEAD_DIM  # 512
B, T = 1, 2048

N_CORES = 8
P = 128
TC = 512  # token chunk
NCH = T // TC  # 4 chunks
NT = TC // P  # 4 token blocks per chunk
KH = 23  # hidden tiles: 23x128 (last zero-padded, carries the bias u-row)
KHP = KH * P  # 2944
QKV_ROWS = Q_MULT * HEAD_DIM + 2 * HEAD_DIM  # 640
QKV_M = QKV_ROWS // P  # 5
CD = 23  # out-proj c tiles: 22x128 + 1x64
C_SIZES = [128] * 22 + [64]

F32 = mybir.dt.float32
BF16 = mybir.dt.bfloat16
F8E4 = mybir.dt.float8e4
AF = mybir.ActivationFunctionType
ALU = mybir.AluOpType

FP8_OUT = False           # fp8 (DoubleRow) out-projection
ATTN_SCALE = 8.0 if FP8_OUT else 1.0
WO_SCALE = 64.0 if FP8_OUT else 1.0

_CACHE = {}


# ------------------------- host-side preparation -------------------------

def _rope_tables():
    d_half = HEAD_DIM // 2
    freq = ROPE_BASE ** (np.arange(0, HEAD_DIM, 2, dtype=np.float64) / HEAD_DIM)
    concentration = 0.1 * math.log(ROPE_SCALE) + 1.0
    low = d_half * math.log(INIT_CTX / (NTK_BETA * 2 * math.pi)) / math.log(ROPE_BASE)
    high = d_half * math.log(INIT_CTX / (NTK_ALPHA * 2 * math.pi)) / math.log(ROPE_BASE)
    interpolation = 1.0 / (ROPE_SCALE * freq)
    extrapolation = 1.0 / freq
    ramp = (np.arange(d_half, dtype=np.float64) - low) / (high - low)
    mask = 1.0 - np.clip(ramp, 0.0, 1.0)
    inv_freq = interpolation * (1.0 - mask) + extrapolation * mask
    pos = np.arange(T, dtype=np.float64)
    angles = pos[:, None] * inv_freq[None, :]  # [T, 32]
    cos = (np.cos(angles) * concentration).astype(np.float32)
    sin = (np.sin(angles) * concentration).astype(np.float32)
    return cos.T.copy(), sin.T.copy()  # [32, T]


def _perm64():
    # evens then odds within a 64-dim head
    return np.concatenate([np.arange(0, 64, 2), np.arange(1, 64, 2)])


def _host_prepare(x, norm_scale, w_qkv, b_qkv, sinks, w_out, b_out):
    x64 = x[0].astype(np.float64)
    s = np.sqrt((x64 * x64).sum(axis=1) + HIDDEN * RMS_EPS)  # [2048]
    xn = x64 * (math.sqrt(HIDDEN) / s)[:, None]  # pre-rmsnormed, ~N(0,1)
    xf = np.zeros((KHP, T), dtype=np.float32)
    xf[:HIDDEN] = xn.T
    xf[HIDDEN] = 1.0  # u row (bias)
    # chunk-contiguous: [NCH, P, KH, TC]
    xb = np.ascontiguousarray(
        xf.reshape(KH, P, NCH, TC).transpose(2, 1, 0, 3)).astype(ml_dtypes.bfloat16)

    w_eff = (w_qkv * norm_scale[None, :]).astype(np.float64)
    b_eff = b_qkv.astype(np.float64).copy()
    # fold softmax scale into q rows (rope is a rotation; scale commutes)
    w_eff[:Q_DIM] *= SM_SCALE
    b_eff[:Q_DIM] *= SM_SCALE

    perm = _perm64()
    cosT, sinT = _rope_tables()
    sin_signed = np.concatenate([-sinT, sinT], axis=0)  # [64, T]

    # band masks for the two score blocks, repeated over 4 heads
    pidx = np.arange(P)[:, None]
    jidx = np.arange(P)[None, :]
    mask_a = np.tile((jidx >= pidx), (1, 4)).astype(ml_dtypes.bfloat16)  # [128, 512]
    mask_b = np.tile((jidx <= pidx), (1, 4)).astype(ml_dtypes.bfloat16)

    per_core = []
    for g in range(N_CORES):
        rows = []
        for h in range(Q_MULT):  # q heads of this group, rope-permuted
            base = (g * Q_MULT + h) * HEAD_DIM
            rows.append(base + perm)
        rows.append(Q_DIM + g * HEAD_DIM + perm)  # k head, rope-permuted
        rows.append(Q_DIM + KV_DIM + g * HEAD_DIM + np.arange(HEAD_DIM))  # v natural
        rows = np.concatenate(rows)
        wq_g = np.zeros((KHP, QKV_ROWS), dtype=np.float64)
        wq_g[:HIDDEN] = w_eff[rows].T
        wq_g[HIDDEN] = b_eff[rows]
        # partition-major for fat per-m DMAs: [P, M, KH, 128]
        wq_g = np.ascontiguousarray(
            wq_g.reshape(KH, P, QKV_M, P).transpose(1, 2, 0, 3)
        ).astype(ml_dtypes.bfloat16)

        # w_out columns for this group, reordered so attn row r of kk-tile j is:
        #   rows 0:64  -> head j      (hg0)
        #   rows 64:128-> head 4 + j  (hg1)
        wo_local = w_out[:, g * KV_DIM:(g + 1) * KV_DIM].T  # [512, 2880]
        order = []
        for kk in range(4):
            order.extend(range(kk * 64, kk * 64 + 64))
            order.extend(range((4 + kk) * 64, (4 + kk) * 64 + 64))
        wo_g = np.ascontiguousarray(wo_local[order])  # [512, 2880]
        if FP8_OUT:
            wo_g = np.clip(wo_g * WO_SCALE, -240, 240).astype(ml_dtypes.float8_e4m3)
        else:
            wo_g = wo_g.astype(ml_dtypes.bfloat16)

        sexp = np.exp2(sinks[g * Q_MULT:(g + 1) * Q_MULT]).astype(np.float64)
        srow = np.repeat(sexp / ATTN_SCALE, P).reshape(1, -1).astype(np.float32)
        per_core.append({
            "xb": xb,
            "wq": wq_g,
            "wo": wo_g,
            "srow": srow,  # [1, 1024] f32, sink/ATTN_SCALE per head
            "cosT": cosT.astype(ml_dtypes.bfloat16),
            "sinS": sin_signed.astype(ml_dtypes.bfloat16),
            "mask_a": mask_a,
            "mask_b": mask_b,
        })
    return per_core


def _partial_to_full(partial):
    """Device partial [NCH, HIDDEN, TC] -> [HIDDEN, T] float64 (unscaled)."""
    p = np.asarray(partial, dtype=np.float64)
    out = p.transpose(1, 0, 2).reshape(HIDDEN, T)
    return out / (ATTN_SCALE * WO_SCALE)


# ------------------------- device program -------------------------

def build_program():
    nc = bacc.Bacc(None, target_bir_lowering=False)

    xb_d = nc.declare_dram_parameter("xb", [NCH, P, KH, TC], BF16, isOutput=False)
    wq_d = nc.declare_dram_parameter("wq", [P, QKV_M, KH, P], BF16, isOutput=False)
    wo_d = nc.declare_dram_parameter("wo", [KV_DIM, HIDDEN],
                                     F8E4 if FP8_OUT else BF16, isOutput=False)
    srow_d = nc.declare_dram_parameter("srow", [1, 2 * TC], F32, isOutput=False)
    cos_d = nc.declare_dram_parameter("cosT", [32, T], BF16, isOutput=False)
    sin_d = nc.declare_dram_parameter("sinS", [64, T], BF16, isOutput=False)
    maska_d = nc.declare_dram_parameter("mask_a", [P, TC], BF16, isOutput=False)
    maskb_d = nc.declare_dram_parameter("mask_b", [P, TC], BF16, isOutput=False)
    out_d = nc.declare_dram_parameter("partial", [NCH, HIDDEN, TC], BF16, isOutput=True)

    with tile.TileContext(nc) as tc:
        _body(tc, nc, xb_d, wq_d, wo_d, srow_d, cos_d, sin_d,
              maska_d, maskb_d, out_d)
    nc.compile()
    return nc


def _body(tc, nc, xb_d, wq_d, wo_d, srow_d, cos_d, sin_d, maska_d, maskb_d, out_d):
    ATTN_DT = F8E4 if FP8_OUT else BF16
    ctx = contextlib.ExitStack()
    with ctx:
        const = ctx.enter_context(tc.tile_pool(name="const", bufs=1))
        xbf = ctx.enter_context(tc.tile_pool(name="xbf", bufs=2))
        qkvp = ctx.enter_context(tc.tile_pool(name="qkvp", bufs=7))
        qsp = ctx.enter_context(tc.tile_pool(name="qsp", bufs=2))
        krp = ctx.enter_context(tc.tile_pool(name="krp", bufs=2))
        kprevp = ctx.enter_context(tc.tile_pool(name="kprevp", bufs=2))
        tmpp = ctx.enter_context(tc.tile_pool(name="tmpp", bufs=2))
        vnatp = ctx.enter_context(tc.tile_pool(name="vnatp", bufs=10))
        expp = ctx.enter_context(tc.tile_pool(name="expp", bufs=4))
        probp = ctx.enter_context(tc.tile_pool(name="probp", bufs=6))
        smallp = ctx.enter_context(tc.tile_pool(name="smallp", bufs=3))
        rsbp = ctx.enter_context(tc.tile_pool(name="rsbp", bufs=3))
        attnp = ctx.enter_context(tc.tile_pool(name="attnp", bufs=2))
        outsb = ctx.enter_context(tc.tile_pool(name="outsb", bufs=3))

        ps_sc = ctx.enter_context(tc.tile_pool(name="ps_sc", bufs=4, space="PSUM"))
        ps_av = ctx.enter_context(tc.tile_pool(name="ps_av", bufs=2, space="PSUM"))
        ps_op = ctx.enter_context(tc.tile_pool(name="ps_op", bufs=2, space="PSUM"))

        # ---- prologue DMAs ----
        # scalar queue: wq m-slices then wo; sync queue: x chunk 0 pieces then
        # small consts.  First qkv matmul needs wq m-slice 0 + x piece 0.
        wq_sb = const.tile([P, QKV_M, KH, P], BF16)
        for m in range(QKV_M):
            nc.scalar.dma_start(out=wq_sb[:, m], in_=wq_d[:, m])
        wo_sb = const.tile([P, 4, HIDDEN], ATTN_DT)
        for kk in range(4):
            nc.scalar.dma_start(out=wo_sb[:, kk, :], in_=wo_d[kk * P:(kk + 1) * P, :])

        cos_sb = const.tile([P, T], BF16)
        sin_sb = const.tile([P, T], BF16)
        srow_sb = const.tile([1, Q_MULT, P], F32)
        mask_a = const.tile([P, TC], BF16)
        mask_b = const.tile([P, TC], BF16)

        def emit_small_dmas():
            for b in range(4):
                nc.sync.dma_start(out=cos_sb[32 * b:32 * (b + 1), :], in_=cos_d[:])
            for b in range(2):
                nc.sync.dma_start(out=sin_sb[64 * b:64 * (b + 1), :], in_=sin_d[:])
            nc.sync.dma_start(out=srow_sb[:], in_=srow_d[:])
            nc.sync.dma_start(out=mask_a[:], in_=maska_d[:])
            nc.sync.dma_start(out=mask_b[:], in_=maskb_d[:])

        # ---------------- emitters ----------------

        def make_qkv(ch):
            """Returns (dma_fn, [m_fn x5], tiles); m_fn emits 23 matmuls+drain."""
            state = {}

            def dma_fn():
                xall = xbf.tile([P, KH, TC], BF16, tag="xk", name=f"xall_{ch}")
                for k0, k1 in ((0, 6), (6, 12), (12, 18), (18, KH)):
                    nc.sync.dma_start(out=xall[:, k0:k1, :],
                                      in_=xb_d[ch, :, k0:k1, :])
                state["x"] = xall

            tiles = [None] * QKV_M

            def make_m(m):
                def m_fn():
                    xall = state["x"]
                    pq = ps_sc.tile([P, TC], F32, tag="sc", name=f"pq_{ch}_{m}")
                    for k in range(KH):
                        nc.tensor.matmul(pq[:], wq_sb[:, m, k, :],
                                         xall[:, k, :],
                                         start=(k == 0), stop=(k == KH - 1))
                    qm = qkvp.tile([P, TC], BF16, tag="qkv", name=f"qm_{ch}_{m}")
                    nc.scalar.activation(qm[:], pq[:], AF.Copy)
                    tiles[m] = qm
                return m_fn

            return dma_fn, [make_m(m) for m in range(QKV_M)], tiles

        def make_rope(ch, qkv_tiles):
            """Per-m rope piece emitters (order 4,0,1,2,3); returns (qs, kr, fns)."""
            t0 = ch * TC
            qs = qsp.tile([64, Q_MULT, TC], BF16, tag="qs", name=f"qs_{ch}")
            kr = krp.tile([64, TC], BF16, tag="kr", name=f"kr_{ch}")

            def make_piece(m):
                def piece():
                    rows = P if m < 4 else HEAD_DIM
                    src = qkv_tiles[m]
                    sw = tmpp.tile([P, TC], BF16, tag="sw", bufs=1,
                                   name=f"sw_{ch}_{m}")
                    for b in range(rows // 64):
                        nc.vector.tensor_copy(sw[64 * b:64 * b + 32, :],
                                              src[64 * b + 32:64 * b + 64, :])
                        nc.vector.tensor_copy(sw[64 * b + 32:64 * b + 64, :],
                                              src[64 * b:64 * b + 32, :])
                    t1 = tmpp.tile([P, TC], BF16, tag="t1", bufs=1,
                                   name=f"t1_{ch}_{m}")
                    nc.vector.tensor_mul(t1[0:rows, :], src[0:rows, :],
                                         cos_sb[0:rows, t0:t0 + TC])
                    t2 = tmpp.tile([P, TC], BF16, tag="t2", bufs=1,
                                   name=f"t2_{ch}_{m}")
                    nc.vector.tensor_mul(t2[0:rows, :], sw[0:rows, :],
                                         sin_sb[0:rows, t0:t0 + TC])
                    if m < 4:
                        qr = tmpp.tile([P, TC], BF16, tag="qr", bufs=1,
                                       name=f"qr_{ch}_{m}")
                        nc.vector.tensor_add(qr[:], t1[:], t2[:])
                        nc.vector.tensor_copy(qs[:, 2 * m, :], qr[0:64, :])
                        nc.vector.tensor_copy(qs[:, 2 * m + 1, :], qr[64:P, :])
                    else:
                        nc.vector.tensor_add(kr[:], t1[0:64, :], t2[0:64, :])
                return piece

            return qs, kr, [make_piece(m) for m in (4, 0, 1, 2, 3)]

        def emit_vnat(ch, qkv_tiles):
            vn_tiles = []
            for tau in range(NT):
                vn = vnatp.tile([P, P], BF16, tag="vn", name=f"vn_{ch}_{tau}")
                nc.vector.memset(vn[:, HEAD_DIM:P], 0.0)
                nc.vector.memset(vn[:, HEAD_DIM:HEAD_DIM + 1], 1.0)
                nc.sync.dma_start(
                    out=vn[:, 0:HEAD_DIM],
                    in_=qkv_tiles[4][HEAD_DIM:P, tau * P:(tau + 1) * P],
                    transpose=True)
                vn_tiles.append(vn)
            return vn_tiles

        def emit_scores_tau(ch, tau, qs, kr, kprev):
            """Score matmuls + exps + mask-muls for both head groups."""
            tg = ch * NT + tau
            kprev_blk = (kr[:, (tau - 1) * P:tau * P] if tau > 0
                         else (kprev[:, :] if kprev is not None else None))
            prs = []
            ets = []
            for hg in range(2):
                qblk = qs[:, hg * 4:(hg + 1) * 4, tau * P:(tau + 1) * P]
                psc_a = ps_sc.tile([P, TC], F32, tag="sc", name=f"pa_{hg}")
                nc.tensor.matmul(psc_a[:], kr[:, tau * P:(tau + 1) * P], qblk,
                                 start=True, stop=True)
                et_a = expp.tile([P, TC], BF16, tag="et", name=f"ea_{hg}")
                nc.scalar.activation(et_a[:], psc_a[:], AF.Exp)
                psc_b = et_b = None
                if tg > 0:
                    psc_b = ps_sc.tile([P, TC], F32, tag="sc", name=f"pb_{hg}")
                    nc.tensor.matmul(psc_b[:], kprev_blk, qblk,
                                     start=True, stop=True)
                    et_b = expp.tile([P, TC], BF16, tag="et", name=f"eb_{hg}")
                    nc.scalar.activation(et_b[:], psc_b[:], AF.Exp)
                ets.append((et_a, et_b))
            for hg in range(2):
                et_a, et_b = ets[hg]
                pr_a = probp.tile([P, TC], BF16, tag="pr", name=f"fa_{hg}")
                nc.vector.tensor_mul(pr_a[:], et_a[:], mask_a[:])
                pr_b = None
                if et_b is not None:
                    pr_b = probp.tile([P, TC], BF16, tag="pr", name=f"fb_{hg}")
                    nc.vector.tensor_mul(pr_b[:], et_b[:], mask_b[:])
                prs.append((pr_a, pr_b))
            return prs

        def emit_pav_tau(ch, tau, prs, vn_tiles, attn_t, vnat_prev):
            """prob@v matmuls + normalize chain for both head groups."""
            tg = ch * NT + tau
            vprev_blk = vn_tiles[tau - 1] if tau > 0 else vnat_prev
            pavs = []
            for hg in range(2):
                pr_a, pr_b = prs[hg]
                pav = ps_av.tile([P, NT, P], F32, tag="av", name=f"pv_{hg}")
                nc.tensor.matmul(pav[:], vn_tiles[tau][:], pr_a[:],
                                 start=True, stop=(tg == 0))
                if tg > 0:
                    nc.tensor.matmul(pav[:], vprev_blk[:], pr_b[:],
                                     start=False, stop=True)
                pavs.append(pav)
            rrs = []
            for hg in range(2):
                den = smallp.tile([1, NT, P], F32, tag="den", name=f"dn_{hg}")
                nc.vector.scalar_tensor_tensor(
                    den[:], pavs[hg][64:65, :, :], 1.0 / ATTN_SCALE,
                    srow_sb[0:1, hg * 4:(hg + 1) * 4, :],
                    op0=ALU.mult, op1=ALU.add)
                rr = smallp.tile([1, NT, P], F32, tag="rr", name=f"rr_{hg}")
                nc.vector.reciprocal_approx_fast(rr[:], den[:])
                rrs.append(rr)
            rsbs = []
            for hg in range(2):
                rsb = rsbp.tile([HEAD_DIM, NT, P], F32, tag="rsb", name=f"rs_{hg}")
                nc.gpsimd.partition_broadcast(rsb[:], rrs[hg][:])
                rsbs.append(rsb)
            for hg in range(2):
                nc.vector.tensor_mul(
                    attn_t[hg * HEAD_DIM:(hg + 1) * HEAD_DIM, :, tau * P:(tau + 1) * P],
                    pavs[hg][0:HEAD_DIM, :, :], rsbs[hg][:])

        def make_op(ch, attn_t):
            """23 out-proj c-tile emitters for chunk ch."""
            def make_c(c, ci):
                cs = C_SIZES[c]

                def c_fn():
                    po = ps_op.tile([P, TC], F32, tag="op", name=f"po_{ch}_{c}")
                    if FP8_OUT:
                        for k2 in range(2):
                            nc.tensor.matmul(
                                po[0:cs, :],
                                wo_sb[:, 2 * k2:2 * k2 + 2, c * P:c * P + cs],
                                attn_t[:, 2 * k2:2 * k2 + 2, :],
                                start=(k2 == 0), stop=(k2 == 1),
                                perf_mode=mybir.MatmulPerfMode.DoubleRow)
                    else:
                        for kk in range(4):
                            nc.tensor.matmul(
                                po[0:cs, :], wo_sb[:, kk, c * P:c * P + cs],
                                attn_t[:, kk, :],
                                start=(kk == 0), stop=(kk == 3))
                    ot = outsb.tile([P, TC], BF16, tag="ot", name=f"ot_{ch}_{c}")
                    # gpsimd cannot read PSUM; split drains scalar/vector
                    if ci % 2 == 0:
                        nc.scalar.activation(ot[0:cs, :], po[0:cs, :], AF.Copy)
                    else:
                        nc.vector.tensor_copy(ot[0:cs, :], po[0:cs, :])
                    nc.sync.dma_start(out=out_d[ch, c * P:c * P + cs, :],
                                      in_=ot[0:cs, :])
                return c_fn

            return [make_c(c, ci) for ci, c in enumerate(range(CD))]

        # ---------------- schedule ----------------
        # qkv m-tiles are emitted in order (4,0,1,2,3) so the k/v tile (m=4)
        # drains first: rope-k and the v transposes unblock early.
        M_ORDER = (4, 0, 1, 2, 3)

        dma0, qkv_m_fns, qkv_tiles_cur = make_qkv(0)
        dma0()
        emit_small_dmas()
        qs_c, kr_c, rope_c = make_rope(0, qkv_tiles_cur)
        vn_c = None
        for i, m in enumerate(M_ORDER):
            qkv_m_fns[m]()
            rope_c[i]()
            if m == 4:
                vn_c = emit_vnat(0, qkv_tiles_cur)

        kprev = None
        vnat_prev = None
        op_prev = []       # out-proj emitters for chunk ch-1

        for ch in range(NCH):
            attn_t = attnp.tile([P, NT, TC], ATTN_DT, tag="attn", name=f"attn_{ch}")

            if ch + 1 < NCH:
                dma_n, qkv_n_fns, qkv_tiles_next = make_qkv(ch + 1)
                dma_n()  # start the next x load before out-DMAs queue up
                qs_n, kr_n, rope_n = make_rope(ch + 1, qkv_tiles_next)
            else:
                qkv_n_fns, qkv_tiles_next = [], None
                qs_n = kr_n = rope_n = None
            vn_n = None

            # out-proj of ch-1 split around each tau's work
            opq = list(op_prev)

            def pop_ops(n):
                for fn in opq[:n]:
                    fn()
                del opq[:n]

            for tau in range(NT):
                pop_ops(3)
                prs = emit_scores_tau(ch, tau, qs_c, kr_c, kprev)
                if qkv_n_fns:
                    qkv_n_fns[M_ORDER[tau]]()
                else:
                    pop_ops(3)
                emit_pav_tau(ch, tau, prs, vn_c, attn_t, vnat_prev)
                if rope_n is not None:
                    rope_n[tau]()
                    if tau == 0:
                        vn_n = emit_vnat(ch + 1, qkv_tiles_next)
                pop_ops(3 if tau < NT - 1 else 23)
                if tau == NT - 1 and qkv_n_fns:
                    qkv_n_fns[M_ORDER[4]]()
                    rope_n[4]()

            kprev_t = kprevp.tile([HEAD_DIM, P], BF16, tag="kp", name=f"kp_{ch}")
            nc.vector.tensor_copy(kprev_t[:], kr_c[:, TC - P:TC])
            kprev = kprev_t
            vnat_prev = vn_c[NT - 1]
            op_prev = make_op(ch, attn_t)
            qs_c, kr_c, vn_c = qs_n, kr_n, vn_n
            qkv_tiles_cur = qkv_tiles_next

        # epilogue: out-proj of the last chunk
        for fn in op_prev:
            fn()


# ------------------------- entry point -------------------------

def _get_program():
    if "nc" not in _CACHE:
        _CACHE["nc"] = build_program()
    return _CACHE["nc"]


def run_cores(inputs, trace=False):
    per_core = _host_prepare(**inputs)
    nc = _get_program()
    res = bass_utils.run_bass_kernel_spmd(
        nc, per_core, core_ids=list(range(N_CORES)), trace=trace,
    )
    return res


def kernel(**inputs):
    res = run_cores(inputs)
    acc = np.zeros((HIDDEN, T), dtype=np.float64)
    for r in res.results:
        acc += _partial_to_full(r["partial"])
    out = acc.T + inputs["x"][0].astype(np.float64) + inputs["b_out"][None, :].astype(np.float64)
    return out[None].astype(np.float32)


# revision 11
# speedup vs baseline: 2.1763x; 1.0622x over previous
"""Trainium2 Bass kernel for nn_AttentionBlock (sliding-window attention block).

Distribution: tensor-parallel over the 8 KV head groups (one group per core).
Each core computes qkv^T for its group (640 rows x 2048 tokens), windowed
attention for its 8 q-heads, and a partial output projection; host sums the
8 partials and adds x + b_out.

Device layout is feature-major: features on partitions, tokens on the free
dim.  Host pre-transposes and pre-rmsnorms x (bf16) and the weights.

Key structure (v3):
- x shipped pre-normalized (x*sqrt(H)/s); bias enters via a ones u-row.
- xb chunk-contiguous in DRAM ([NCH, P, KH, TC]), loaded in 4 pieces per
  chunk so the qkv k-loop can start before the full chunk lands.
- wq shipped partition-major ([P, M, KH, 128]) so each m-slice loads with
  one fat DMA; k-tiles zero-padded to 128 rows.
- softmax denominator: sink added on DVE (scalar_tensor_tensor), reciprocal
  on DVE, partition-broadcast on GpSimd -- no PE rank-1 matmuls.
- attention output tile is [128, 4, TC] with head h -> (kk=h%4, half=h//4)
  so the normalize multiply writes it directly (no layout copies); w_out is
  host-reordered to match.
- per-tau PE queue: [scores][op 3c][qkv m][pav][op 3c] so the PE never
  waits on the exp/mask/normalize chains; out-proj of chunk ch-1 and qkv of
  ch+1 fill all gaps.  GpSimd runs only partition_broadcast + rope copies
  (it is slow and must stay off the critical path).
- optional fp8 out-projection (DoubleRow) controlled by FP8_OUT.
"""

import contextlib
import math

import numpy as np
import ml_dtypes

import concourse.bass as bass
import concourse.mybir as mybir
import concourse.tile as tile
from concourse import bacc, bass_utils

# ---- problem config (hardcoded from the reference) ----
HIDDEN = 2880
HEAD_DIM = 64
N_HEADS = 64
N_KV = 8
Q_MULT = N_HEADS // N_KV  # 8
SLIDING_WINDOW = 128
ROPE_BASE = 150000.0
ROPE_SCALE = 32.0
NTK_ALPHA = 1.0
NTK_BETA = 32.0
INIT_CTX = 4096
RMS_EPS = 1e-5
SM_SCALE = 1.0 / math.sqrt(HEAD_DIM)
Q_DIM = N_HEADS * HEAD_DIM  # 4096
KV_DIM = N_KV * HEAD_DIM  # 512
B, T = 1, 2048

N_CORES = 8
P = 128
TC = 512  # token chunk
NCH = T // TC  # 4 chunks
NT = TC // P  # 4 token blocks per chunk
KH = 23  # hidden tiles: 23x128 (last zero-padded, carries the bias u-row)
KHP = KH * P  # 2944
QKV_ROWS = Q_MULT * HEAD_DIM + 2 * HEAD_DIM  # 640
QKV_M = QKV_ROWS // P  # 5
CD = 23  # out-proj c tiles: 22x128 + 1x64
C_SIZES = [128] * 22 + [64]

F32 = mybir.dt.float32
BF16 = mybir.dt.bfloat16
F8E4 = mybir.dt.float8e4
AF = mybir.ActivationFunctionType
ALU = mybir.AluOpType

FP8_OUT = True            # fp8 (DoubleRow) out-projection
ATTN_SCALE = 8.0 if FP8_OUT else 1.0
WO_SCALE = 64.0 if FP8_OUT else 1.0

_CACHE = {}


# ------------------------- host-side preparation -------------------------

def _rope_tables():
    d_half = HEAD_DIM // 2
    freq = ROPE_BASE ** (np.arange(0, HEAD_DIM, 2, dtype=np.float64) / HEAD_DIM)
    concentration = 0.1 * math.log(ROPE_SCALE) + 1.0
    low = d_half * math.log(INIT_CTX / (NTK_BETA * 2 * math.pi)) / math.log(ROPE_BASE)
    high = d_half * math.log(INIT_CTX / (NTK_ALPHA * 2 * math.pi)) / math.log(ROPE_BASE)
    interpolation = 1.0 / (ROPE_SCALE * freq)
    extrapolation = 1.0 / freq
    ramp = (np.arange(d_half, dtype=np.float64) - low) / (high - low)
    mask = 1.0 - np.clip(ramp, 0.0, 1.0)
    inv_freq = interpolation * (1.0 - mask) + extrapolation * mask
    pos = np.arange(T, dtype=np.float64)
    angles = pos[:, None] * inv_freq[None, :]  # [T, 32]
    cos = (np.cos(angles) * concentration).astype(np.float32)
    sin = (np.sin(angles) * concentration).astype(np.float32)
    return cos.T.copy(), sin.T.copy()  # [32, T]


def _perm64():
    # evens then odds within a 64-dim head
    return np.concatenate([np.arange(0, 64, 2), np.arange(1, 64, 2)])


def _host_prepare(x, norm_scale, w_qkv, b_qkv, sinks, w_out, b_out):
    x64 = x[0].astype(np.float64)
    s = np.sqrt((x64 * x64).sum(axis=1) + HIDDEN * RMS_EPS)  # [2048]
    xn = x64 * (math.sqrt(HIDDEN) / s)[:, None]  # pre-rmsnormed, ~N(0,1)
    xf = np.zeros((KHP, T), dtype=np.float32)
    xf[:HIDDEN] = xn.T
    xf[HIDDEN] = 1.0  # u row (bias)
    # chunk-contiguous: [NCH, P, KH, TC]
    xb = np.ascontiguousarray(
        xf.reshape(KH, P, NCH, TC).transpose(2, 1, 0, 3)).astype(ml_dtypes.bfloat16)

    w_eff = (w_qkv * norm_scale[None, :]).astype(np.float64)
    b_eff = b_qkv.astype(np.float64).copy()
    # fold softmax scale into q rows (rope is a rotation; scale commutes)
    w_eff[:Q_DIM] *= SM_SCALE
    b_eff[:Q_DIM] *= SM_SCALE

    perm = _perm64()
    cosT, sinT = _rope_tables()
    sin_signed = np.concatenate([-sinT, sinT], axis=0)  # [64, T]

    # band masks for the two score blocks, repeated over 4 heads
    pidx = np.arange(P)[:, None]
    jidx = np.arange(P)[None, :]
    mask_a = np.tile((jidx >= pidx), (1, 4)).astype(ml_dtypes.bfloat16)  # [128, 512]
    mask_b = np.tile((jidx <= pidx), (1, 4)).astype(ml_dtypes.bfloat16)

    per_core = []
    for g in range(N_CORES):
        rows = []
        for h in range(Q_MULT):  # q heads of this group, rope-permuted
            base = (g * Q_MULT + h) * HEAD_DIM
            rows.append(base + perm)
        rows.append(Q_DIM + g * HEAD_DIM + perm)  # k head, rope-permuted
        rows.append(Q_DIM + KV_DIM + g * HEAD_DIM + np.arange(HEAD_DIM))  # v natural
        rows = np.concatenate(rows)
        wq_g = np.zeros((KHP, QKV_ROWS), dtype=np.float64)
        wq_g[:HIDDEN] = w_eff[rows].T
        wq_g[HIDDEN] = b_eff[rows]
        # partition-major for fat per-m DMAs: [P, M, KH, 128]
        wq_g = np.ascontiguousarray(
            wq_g.reshape(KH, P, QKV_M, P).transpose(1, 2, 0, 3)
        ).astype(ml_dtypes.bfloat16)

        # w_out columns for this group, reordered so attn row r of kk-tile j is:
        #   rows 0:64  -> head j      (hg0)
        #   rows 64:128-> head 4 + j  (hg1)
        wo_local = w_out[:, g * KV_DIM:(g + 1) * KV_DIM].T  # [512, 2880]
        order = []
        for kk in range(4):
            order.extend(range(kk * 64, kk * 64 + 64))
            order.extend(range((4 + kk) * 64, (4 + kk) * 64 + 64))
        wo_g = np.ascontiguousarray(wo_local[order])  # [512, 2880]
        if FP8_OUT:
            wo_g = np.clip(wo_g * WO_SCALE, -240, 240).astype(ml_dtypes.float8_e4m3)
        else:
            wo_g = wo_g.astype(ml_dtypes.bfloat16)

        sexp = np.exp2(sinks[g * Q_MULT:(g + 1) * Q_MULT]).astype(np.float64)
        srow = np.repeat(sexp / ATTN_SCALE, P).reshape(1, -1).astype(np.float32)
        per_core.append({
            "xb": xb,
            "wq": wq_g,
            "wo": wo_g,
            "srow": srow,  # [1, 1024] f32, sink/ATTN_SCALE per head
            "cosT": cosT.astype(ml_dtypes.bfloat16),
            "sinS": sin_signed.astype(ml_dtypes.bfloat16),
            "mask_a": mask_a,
            "mask_b": mask_b,
        })
    return per_core


def _partial_to_full(partial):
    """Device partial [NCH, HIDDEN, TC] -> [HIDDEN, T] float64 (unscaled)."""
    p = np.asarray(partial, dtype=np.float64)
    out = p.transpose(1, 0, 2).reshape(HIDDEN, T)
    return out / (ATTN_SCALE * WO_SCALE)


# ------------------------- device program -------------------------

def build_program():
    nc = bacc.Bacc(None, target_bir_lowering=False)

    xb_d = nc.declare_dram_parameter("xb", [NCH, P, KH, TC], BF16, isOutput=False)
    wq_d = nc.declare_dram_parameter("wq", [P, QKV_M, KH, P], BF16, isOutput=False)
    wo_d = nc.declare_dram_parameter("wo", [KV_DIM, HIDDEN],
                                     F8E4 if FP8_OUT else BF16, isOutput=False)
    srow_d = nc.declare_dram_parameter("srow", [1, 2 * TC], F32, isOutput=False)
    cos_d = nc.declare_dram_parameter("cosT", [32, T], BF16, isOutput=False)
    sin_d = nc.declare_dram_parameter("sinS", [64, T], BF16, isOutput=False)
    maska_d = nc.declare_dram_parameter("mask_a", [P, TC], BF16, isOutput=False)
    maskb_d = nc.declare_dram_parameter("mask_b", [P, TC], BF16, isOutput=False)
    out_d = nc.declare_dram_parameter("partial", [NCH, HIDDEN, TC], BF16, isOutput=True)

    with tile.TileContext(nc) as tc:
        _body(tc, nc, xb_d, wq_d, wo_d, srow_d, cos_d, sin_d,
              maska_d, maskb_d, out_d)
    nc.compile()
    return nc


def _body(tc, nc, xb_d, wq_d, wo_d, srow_d, cos_d, sin_d, maska_d, maskb_d, out_d):
    ATTN_DT = F8E4 if FP8_OUT else BF16
    ctx = contextlib.ExitStack()
    with ctx:
        const = ctx.enter_context(tc.tile_pool(name="const", bufs=1))
        xbf = ctx.enter_context(tc.tile_pool(name="xbf", bufs=2))
        qkvp = ctx.enter_context(tc.tile_pool(name="qkvp", bufs=7))
        qsp = ctx.enter_context(tc.tile_pool(name="qsp", bufs=2))
        krp = ctx.enter_context(tc.tile_pool(name="krp", bufs=2))
        kprevp = ctx.enter_context(tc.tile_pool(name="kprevp", bufs=2))
        tmpp = ctx.enter_context(tc.tile_pool(name="tmpp", bufs=2))
        vnatp = ctx.enter_context(tc.tile_pool(name="vnatp", bufs=10))
        expp = ctx.enter_context(tc.tile_pool(name="expp", bufs=4))
        probp = ctx.enter_context(tc.tile_pool(name="probp", bufs=6))
        smallp = ctx.enter_context(tc.tile_pool(name="smallp", bufs=3))
        rsbp = ctx.enter_context(tc.tile_pool(name="rsbp", bufs=3))
        attnp = ctx.enter_context(tc.tile_pool(name="attnp", bufs=2))
        outsb = ctx.enter_context(tc.tile_pool(name="outsb", bufs=3))

        ps_sc = ctx.enter_context(tc.tile_pool(name="ps_sc", bufs=4, space="PSUM"))
        ps_av = ctx.enter_context(tc.tile_pool(name="ps_av", bufs=2, space="PSUM"))
        ps_op = ctx.enter_context(tc.tile_pool(name="ps_op", bufs=2, space="PSUM"))

        # ---- prologue DMAs ----
        # scalar queue: wq m-slices then wo; sync queue: x chunk 0 pieces then
        # small consts.  First qkv matmul needs wq m-slice 0 + x piece 0.
        wq_sb = const.tile([P, QKV_M, KH, P], BF16)
        for m in (4, 0, 1, 2, 3):  # m=4 first: it is the first matmul emitted
            nc.scalar.dma_start(out=wq_sb[:, m], in_=wq_d[:, m])
        wo_sb = const.tile([P, 4, HIDDEN], ATTN_DT)
        for kk in range(4):
            nc.scalar.dma_start(out=wo_sb[:, kk, :], in_=wo_d[kk * P:(kk + 1) * P, :])

        cos_sb = const.tile([P, T], BF16)
        sin_sb = const.tile([P, T], BF16)
        srow_sb = const.tile([1, Q_MULT, P], F32)
        mask_a = const.tile([P, TC], BF16)
        mask_b = const.tile([P, TC], BF16)

        def emit_small_dmas():
            for b in range(4):
                nc.sync.dma_start(out=cos_sb[32 * b:32 * (b + 1), :], in_=cos_d[:])
            for b in range(2):
                nc.sync.dma_start(out=sin_sb[64 * b:64 * (b + 1), :], in_=sin_d[:])
            nc.sync.dma_start(out=srow_sb[:], in_=srow_d[:])
            nc.sync.dma_start(out=mask_a[:], in_=maska_d[:])
            nc.sync.dma_start(out=mask_b[:], in_=maskb_d[:])

        # ---------------- emitters ----------------

        def make_qkv(ch):
            """Returns (dma_fn, [m_fn x5], tiles); m_fn emits 23 matmuls+drain."""
            state = {}

            def dma_fn():
                xall = xbf.tile([P, KH, TC], BF16, tag="xk", name=f"xall_{ch}")
                for k0, k1 in ((0, 6), (6, 12), (12, 18), (18, KH)):
                    nc.sync.dma_start(out=xall[:, k0:k1, :],
                                      in_=xb_d[ch, :, k0:k1, :])
                state["x"] = xall

            tiles = [None] * QKV_M

            def make_m(m):
                def m_fn():
                    xall = state["x"]
                    pq = ps_sc.tile([P, TC], F32, tag="sc", name=f"pq_{ch}_{m}")
                    for k in range(KH):
                        nc.tensor.matmul(pq[:], wq_sb[:, m, k, :],
                                         xall[:, k, :],
                                         start=(k == 0), stop=(k == KH - 1))
                    qm = qkvp.tile([P, TC], BF16, tag="qkv", name=f"qm_{ch}_{m}")
                    nc.scalar.activation(qm[:], pq[:], AF.Copy)
                    tiles[m] = qm
                return m_fn

            return dma_fn, [make_m(m) for m in range(QKV_M)], tiles

        def make_rope(ch, qkv_tiles):
            """Per-m rope piece emitters (order 4,0,1,2,3); returns (qs, kr, fns)."""
            t0 = ch * TC
            qs = qsp.tile([64, Q_MULT, TC], BF16, tag="qs", name=f"qs_{ch}")
            kr = krp.tile([64, TC], BF16, tag="kr", name=f"kr_{ch}")

            def make_piece(m):
                def piece():
                    rows = P if m < 4 else HEAD_DIM
                    src = qkv_tiles[m]
                    sw = tmpp.tile([P, TC], BF16, tag="sw", bufs=1,
                                   name=f"sw_{ch}_{m}")
                    for b in range(rows // 64):
                        nc.vector.tensor_copy(sw[64 * b:64 * b + 32, :],
                                              src[64 * b + 32:64 * b + 64, :])
                        nc.vector.tensor_copy(sw[64 * b + 32:64 * b + 64, :],
                                              src[64 * b:64 * b + 32, :])
                    t1 = tmpp.tile([P, TC], BF16, tag="t1", bufs=1,
                                   name=f"t1_{ch}_{m}")
                    nc.vector.tensor_mul(t1[0:rows, :], src[0:rows, :],
                                         cos_sb[0:rows, t0:t0 + TC])
                    t2 = tmpp.tile([P, TC], BF16, tag="t2", bufs=1,
                                   name=f"t2_{ch}_{m}")
                    nc.vector.tensor_mul(t2[0:rows, :], sw[0:rows, :],
                                         sin_sb[0:rows, t0:t0 + TC])
                    if m < 4:
                        qr = tmpp.tile([P, TC], BF16, tag="qr", bufs=1,
                                       name=f"qr_{ch}_{m}")
                        nc.vector.tensor_add(qr[:], t1[:], t2[:])
                        nc.vector.tensor_copy(qs[:, 2 * m, :], qr[0:64, :])
                        nc.vector.tensor_copy(qs[:, 2 * m + 1, :], qr[64:P, :])
                    else:
                        nc.vector.tensor_add(kr[:], t1[0:64, :], t2[0:64, :])
                return piece

            return qs, kr, [make_piece(m) for m in (4, 0, 1, 2, 3)]

        def emit_vnat(ch, qkv_tiles):
            vn_tiles = []
            for tau in range(NT):
                vn = vnatp.tile([P, P], BF16, tag="vn", name=f"vn_{ch}_{tau}")
                nc.vector.memset(vn[:, HEAD_DIM:P], 0.0)
                nc.vector.memset(vn[:, HEAD_DIM:HEAD_DIM + 1], 1.0)
                nc.sync.dma_start(
                    out=vn[:, 0:HEAD_DIM],
                    in_=qkv_tiles[4][HEAD_DIM:P, tau * P:(tau + 1) * P],
                    transpose=True)
                vn_tiles.append(vn)
            return vn_tiles

        def emit_scores_tau(ch, tau, qs, kr, kprev):
            """Score matmuls + exps + mask-muls for both head groups."""
            tg = ch * NT + tau
            kprev_blk = (kr[:, (tau - 1) * P:tau * P] if tau > 0
                         else (kprev[:, :] if kprev is not None else None))
            prs = []
            ets = []
            for hg in range(2):
                qblk = qs[:, hg * 4:(hg + 1) * 4, tau * P:(tau + 1) * P]
                psc_a = ps_sc.tile([P, TC], F32, tag="sc", name=f"pa_{hg}")
                nc.tensor.matmul(psc_a[:], kr[:, tau * P:(tau + 1) * P], qblk,
                                 start=True, stop=True)
                et_a = expp.tile([P, TC], BF16, tag="et", name=f"ea_{hg}")
                nc.scalar.activation(et_a[:], psc_a[:], AF.Exp)
                psc_b = et_b = None
                if tg > 0:
                    psc_b = ps_sc.tile([P, TC], F32, tag="sc", name=f"pb_{hg}")
                    nc.tensor.matmul(psc_b[:], kprev_blk, qblk,
                                     start=True, stop=True)
                    et_b = expp.tile([P, TC], BF16, tag="et", name=f"eb_{hg}")
                    nc.scalar.activation(et_b[:], psc_b[:], AF.Exp)
                ets.append((et_a, et_b))
            for hg in range(2):
                et_a, et_b = ets[hg]
                pr_a = probp.tile([P, TC], BF16, tag="pr", name=f"fa_{hg}")
                nc.vector.tensor_mul(pr_a[:], et_a[:], mask_a[:])
                pr_b = None
                if et_b is not None:
                    pr_b = probp.tile([P, TC], BF16, tag="pr", name=f"fb_{hg}")
                    nc.vector.tensor_mul(pr_b[:], et_b[:], mask_b[:])
                prs.append((pr_a, pr_b))
            return prs

        def emit_pav_tau(ch, tau, prs, vn_tiles, attn_t, vnat_prev):
            """prob@v matmuls + normalize chain for both head groups."""
            tg = ch * NT + tau
            vprev_blk = vn_tiles[tau - 1] if tau > 0 else vnat_prev
            pavs = []
            for hg in range(2):
                pr_a, pr_b = prs[hg]
                pav = ps_av.tile([P, NT, P], F32, tag="av", name=f"pv_{hg}")
                nc.tensor.matmul(pav[:], vn_tiles[tau][:], pr_a[:],
                                 start=True, stop=(tg == 0))
                if tg > 0:
                    nc.tensor.matmul(pav[:], vprev_blk[:], pr_b[:],
                                     start=False, stop=True)
                pavs.append(pav)
            rrs = []
            for hg in range(2):
                den = smallp.tile([1, NT, P], F32, tag="den", name=f"dn_{hg}")
                nc.vector.scalar_tensor_tensor(
                    den[:], pavs[hg][64:65, :, :], 1.0 / ATTN_SCALE,
                    srow_sb[0:1, hg * 4:(hg + 1) * 4, :],
                    op0=ALU.mult, op1=ALU.add)
                rr = smallp.tile([1, NT, P], F32, tag="rr", name=f"rr_{hg}")
                nc.vector.reciprocal_approx_fast(rr[:], den[:])
                rrs.append(rr)
            rsbs = []
            for hg in range(2):
                rsb = rsbp.tile([HEAD_DIM, NT, P], F32, tag="rsb", name=f"rs_{hg}")
                nc.gpsimd.partition_broadcast(rsb[:], rrs[hg][:])
                rsbs.append(rsb)
            for hg in range(2):
                nc.vector.tensor_mul(
                    attn_t[hg * HEAD_DIM:(hg + 1) * HEAD_DIM, :, tau * P:(tau + 1) * P],
                    pavs[hg][0:HEAD_DIM, :, :], rsbs[hg][:])

        def make_op(ch, attn_t):
            """23 out-proj c-tile emitters for chunk ch."""
            def make_c(c, ci):
                cs = C_SIZES[c]

                def c_fn():
                    po = ps_op.tile([P, TC], F32, tag="op", name=f"po_{ch}_{c}")
                    if FP8_OUT:
                        for k2 in range(2):
                            nc.tensor.matmul(
                                po[0:cs, :],
                                wo_sb[:, 2 * k2:2 * k2 + 2, c * P:c * P + cs],
                                attn_t[:, 2 * k2:2 * k2 + 2, :],
                                start=(k2 == 0), stop=(k2 == 1),
                                perf_mode=mybir.MatmulPerfMode.DoubleRow)
                    else:
                        for kk in range(4):
                            nc.tensor.matmul(
                                po[0:cs, :], wo_sb[:, kk, c * P:c * P + cs],
                                attn_t[:, kk, :],
                                start=(kk == 0), stop=(kk == 3))
                    ot = outsb.tile([P, TC], BF16, tag="ot", name=f"ot_{ch}_{c}")
                    # gpsimd cannot read PSUM; split drains scalar/vector
                    if ci % 2 == 0:
                        nc.scalar.activation(ot[0:cs, :], po[0:cs, :], AF.Copy)
                    else:
                        nc.vector.tensor_copy(ot[0:cs, :], po[0:cs, :])
                    nc.sync.dma_start(out=out_d[ch, c * P:c * P + cs, :],
                                      in_=ot[0:cs, :])
                return c_fn

            return [make_c(c, ci) for ci, c in enumerate(range(CD))]

        # ---------------- schedule ----------------
        # qkv m-tiles are emitted in order (4,0,1,2,3) so the k/v tile (m=4)
        # drains first: rope-k and the v transposes unblock early.
        M_ORDER = (4, 0, 1, 2, 3)

        dma0, qkv_m_fns, qkv_tiles_cur = make_qkv(0)
        dma0()
        emit_small_dmas()
        qs_c, kr_c, rope_c = make_rope(0, qkv_tiles_cur)
        vn_c = None
        for i, m in enumerate(M_ORDER):
            qkv_m_fns[m]()
            rope_c[i]()
            if m == 4:
                vn_c = emit_vnat(0, qkv_tiles_cur)

        kprev = None
        vnat_prev = None
        op_prev = []       # out-proj emitters for chunk ch-1

        for ch in range(NCH):
            attn_t = attnp.tile([P, NT, TC], ATTN_DT, tag="attn", name=f"attn_{ch}")

            if ch + 1 < NCH:
                dma_n, qkv_n_fns, qkv_tiles_next = make_qkv(ch + 1)
                dma_n()  # start the next x load before out-DMAs queue up
                qs_n, kr_n, rope_n = make_rope(ch + 1, qkv_tiles_next)
            else:
                qkv_n_fns, qkv_tiles_next = [], None
                qs_n = kr_n = rope_n = None
            vn_n = None

            # out-proj of ch-1 split around each tau's work
            opq = list(op_prev)

            def pop_ops(n):
                for fn in opq[:n]:
                    fn()
                del opq[:n]

            for tau in range(NT):
                pop_ops(3)
                prs = emit_scores_tau(ch, tau, qs_c, kr_c, kprev)
                if qkv_n_fns:
                    qkv_n_fns[M_ORDER[tau]]()
                else:
                    pop_ops(3)
                emit_pav_tau(ch, tau, prs, vn_c, attn_t, vnat_prev)
                if rope_n is not None:
                    rope_n[tau]()
                    if tau == 0:
                        vn_n = emit_vnat(ch + 1, qkv_tiles_next)
                pop_ops(3 if tau < NT - 1 else 23)
                if tau == NT - 1 and qkv_n_fns:
                    qkv_n_fns[M_ORDER[4]]()
                    rope_n[4]()

            kprev_t = kprevp.tile([HEAD_DIM, P], BF16, tag="kp", name=f"kp_{ch}")
            nc.vector.tensor_copy(kprev_t[:], kr_c[:, TC - P:TC])
            kprev = kprev_t
            vnat_prev = vn_c[NT - 1]
            op_prev = make_op(ch, attn_t)
            qs_c, kr_c, vn_c = qs_n, kr_n, vn_n
            qkv_tiles_cur = qkv_tiles_next

        # epilogue: out-proj of the last chunk
        for fn in op_prev:
            fn()


# ------------------------- entry point -------------------------

def _get_program():
    if "nc" not in _CACHE:
        _CACHE["nc"] = build_program()
    return _CACHE["nc"]


def run_cores(inputs, trace=False):
    per_core = _host_prepare(**inputs)
    nc = _get_program()
    res = bass_utils.run_bass_kernel_spmd(
        nc, per_core, core_ids=list(range(N_CORES)), trace=trace,
    )
    return res


def kernel(**inputs):
    res = run_cores(inputs)
    acc = np.zeros((HIDDEN, T), dtype=np.float64)
    for r in res.results:
        acc += _partial_to_full(r["partial"])
    out = acc.T + inputs["x"][0].astype(np.float64) + inputs["b_out"][None, :].astype(np.float64)
    return out[None].astype(np.float32)


# revision 14
# speedup vs baseline: 2.2434x; 1.0308x over previous
"""Trainium2 Bass kernel for nn_AttentionBlock (sliding-window attention block).

Distribution: tensor-parallel over the 8 KV head groups (one group per core).
Each core computes qkv^T for its group (640 rows x 2048 tokens), windowed
attention for its 8 q-heads, and a partial output projection; host sums the
8 partials and adds x + b_out.

Device layout is feature-major: features on partitions, tokens on the free
dim.  Host pre-transposes and pre-rmsnorms x (bf16) and the weights.

Key structure (v3):
- x shipped pre-normalized (x*sqrt(H)/s); bias enters via a ones u-row.
- xb chunk-contiguous in DRAM ([NCH, P, KH, TC]), loaded in 4 pieces per
  chunk so the qkv k-loop can start before the full chunk lands.
- wq shipped partition-major ([P, M, KH, 128]) so each m-slice loads with
  one fat DMA; k-tiles zero-padded to 128 rows.
- softmax denominator: sink added on DVE (scalar_tensor_tensor), reciprocal
  on DVE, partition-broadcast on GpSimd -- no PE rank-1 matmuls.
- attention output tile is [128, 4, TC] with head h -> (kk=h%4, half=h//4)
  so the normalize multiply writes it directly (no layout copies); w_out is
  host-reordered to match.
- per-tau PE queue: [scores][op 3c][qkv m][pav][op 3c] so the PE never
  waits on the exp/mask/normalize chains; out-proj of chunk ch-1 and qkv of
  ch+1 fill all gaps.  GpSimd runs only partition_broadcast + rope copies
  (it is slow and must stay off the critical path).
- optional fp8 out-projection (DoubleRow) controlled by FP8_OUT.
"""

import contextlib
import math

import numpy as np
import ml_dtypes

import concourse.bass as bass
import concourse.mybir as mybir
import concourse.tile as tile
from concourse import bacc, bass_utils

# ---- problem config (hardcoded from the reference) ----
HIDDEN = 2880
HEAD_DIM = 64
N_HEADS = 64
N_KV = 8
Q_MULT = N_HEADS // N_KV  # 8
SLIDING_WINDOW = 128
ROPE_BASE = 150000.0
ROPE_SCALE = 32.0
NTK_ALPHA = 1.0
NTK_BETA = 32.0
INIT_CTX = 4096
RMS_EPS = 1e-5
SM_SCALE = 1.0 / math.sqrt(HEAD_DIM)
Q_DIM = N_HEADS * HEAD_DIM  # 4096
KV_DIM = N_KV * HEAD_DIM  # 512
B, T = 1, 2048

N_CORES = 8
P = 128
TC = 512  # token chunk
NCH = T // TC  # 4 chunks
NT = TC // P  # 4 token blocks per chunk
KH = 23  # hidden tiles: 23x128 (last zero-padded, carries the bias u-row)
KHP = KH * P  # 2944
QKV_ROWS = Q_MULT * HEAD_DIM + 2 * HEAD_DIM  # 640
QKV_M = QKV_ROWS // P  # 5
CD = 23  # out-proj c tiles: 22x128 + 1x64
C_SIZES = [128] * 22 + [64]

F32 = mybir.dt.float32
BF16 = mybir.dt.bfloat16
F8E4 = mybir.dt.float8e4
AF = mybir.ActivationFunctionType
ALU = mybir.AluOpType

FP8_OUT = True            # fp8 (DoubleRow) out-projection
ATTN_SCALE = 8.0 if FP8_OUT else 1.0
WO_SCALE = 64.0 if FP8_OUT else 1.0

_CACHE = {}


# ------------------------- host-side preparation -------------------------

def _rope_tables():
    d_half = HEAD_DIM // 2
    freq = ROPE_BASE ** (np.arange(0, HEAD_DIM, 2, dtype=np.float64) / HEAD_DIM)
    concentration = 0.1 * math.log(ROPE_SCALE) + 1.0
    low = d_half * math.log(INIT_CTX / (NTK_BETA * 2 * math.pi)) / math.log(ROPE_BASE)
    high = d_half * math.log(INIT_CTX / (NTK_ALPHA * 2 * math.pi)) / math.log(ROPE_BASE)
    interpolation = 1.0 / (ROPE_SCALE * freq)
    extrapolation = 1.0 / freq
    ramp = (np.arange(d_half, dtype=np.float64) - low) / (high - low)
    mask = 1.0 - np.clip(ramp, 0.0, 1.0)
    inv_freq = interpolation * (1.0 - mask) + extrapolation * mask
    pos = np.arange(T, dtype=np.float64)
    angles = pos[:, None] * inv_freq[None, :]  # [T, 32]
    cos = (np.cos(angles) * concentration).astype(np.float32)
    sin = (np.sin(angles) * concentration).astype(np.float32)
    return cos.T.copy(), sin.T.copy()  # [32, T]


def _perm64():
    # evens then odds within a 64-dim head
    return np.concatenate([np.arange(0, 64, 2), np.arange(1, 64, 2)])


def _host_prepare(x, norm_scale, w_qkv, b_qkv, sinks, w_out, b_out):
    x64 = x[0].astype(np.float64)
    s = np.sqrt((x64 * x64).sum(axis=1) + HIDDEN * RMS_EPS)  # [2048]
    xn = x64 * (math.sqrt(HIDDEN) / s)[:, None]  # pre-rmsnormed, ~N(0,1)
    xf = np.zeros((KHP, T), dtype=np.float32)
    xf[:HIDDEN] = xn.T
    xf[HIDDEN] = 1.0  # u row (bias)
    # chunk-contiguous: [NCH, P, KH, TC]
    xb = np.ascontiguousarray(
        xf.reshape(KH, P, NCH, TC).transpose(2, 1, 0, 3)).astype(ml_dtypes.bfloat16)

    w_eff = (w_qkv * norm_scale[None, :]).astype(np.float64)
    b_eff = b_qkv.astype(np.float64).copy()
    # fold softmax scale into q rows (rope is a rotation; scale commutes)
    w_eff[:Q_DIM] *= SM_SCALE
    b_eff[:Q_DIM] *= SM_SCALE

    perm = _perm64()
    cosT, sinT = _rope_tables()
    sin_signed = np.concatenate([-sinT, sinT], axis=0)  # [64, T]

    # band masks for the two score blocks, repeated over 4 heads
    pidx = np.arange(P)[:, None]
    jidx = np.arange(P)[None, :]
    mask_a = np.tile((jidx >= pidx), (1, 4)).astype(ml_dtypes.bfloat16)  # [128, 512]
    mask_b = np.tile((jidx <= pidx), (1, 4)).astype(ml_dtypes.bfloat16)

    per_core = []
    for g in range(N_CORES):
        rows = []
        for h in range(Q_MULT):  # q heads of this group, rope-permuted
            base = (g * Q_MULT + h) * HEAD_DIM
            rows.append(base + perm)
        rows.append(Q_DIM + g * HEAD_DIM + perm)  # k head, rope-permuted
        rows.append(Q_DIM + KV_DIM + g * HEAD_DIM + np.arange(HEAD_DIM))  # v natural
        rows = np.concatenate(rows)
        wq_g = np.zeros((KHP, QKV_ROWS), dtype=np.float64)
        wq_g[:HIDDEN] = w_eff[rows].T
        wq_g[HIDDEN] = b_eff[rows]
        # partition-major for fat per-m DMAs: [P, M, KH, 128]
        wq_g = np.ascontiguousarray(
            wq_g.reshape(KH, P, QKV_M, P).transpose(1, 2, 0, 3)
        ).astype(ml_dtypes.bfloat16)

        # w_out columns for this group, reordered so attn row r of kk-tile j is:
        #   rows 0:64  -> head j      (hg0)
        #   rows 64:128-> head 4 + j  (hg1)
        wo_local = w_out[:, g * KV_DIM:(g + 1) * KV_DIM].T  # [512, 2880]
        order = []
        for kk in range(4):
            order.extend(range(kk * 64, kk * 64 + 64))
            order.extend(range((4 + kk) * 64, (4 + kk) * 64 + 64))
        wo_g = np.ascontiguousarray(wo_local[order])  # [512, 2880]
        if FP8_OUT:
            wo_g = np.clip(wo_g * WO_SCALE, -240, 240).astype(ml_dtypes.float8_e4m3)
        else:
            wo_g = wo_g.astype(ml_dtypes.bfloat16)

        sexp = np.exp2(sinks[g * Q_MULT:(g + 1) * Q_MULT]).astype(np.float64)
        srow = np.repeat(sexp / ATTN_SCALE, P).reshape(1, -1).astype(np.float32)
        per_core.append({
            "xb": xb,
            "wq": wq_g,
            "wo": wo_g,
            "srow": srow,  # [1, 1024] f32, sink/ATTN_SCALE per head
            "cosT": cosT.astype(ml_dtypes.bfloat16),
            "sinS": sin_signed.astype(ml_dtypes.bfloat16),
            "mask_a": mask_a,
            "mask_b": mask_b,
        })
    return per_core


def _partial_to_full(partial):
    """Device partial [NCH, P, CD, TC] -> [HIDDEN, T] float64 (unscaled)."""
    p = np.asarray(partial, dtype=np.float64)
    # row c*P + p of chunk ch lives at partial[ch, p, c, :]
    out = p.transpose(2, 1, 0, 3).reshape(CD * P, T)[:HIDDEN]
    return out / (ATTN_SCALE * WO_SCALE)


# ------------------------- device program -------------------------

def build_program():
    nc = bacc.Bacc(None, target_bir_lowering=False)

    xb_d = nc.declare_dram_parameter("xb", [NCH, P, KH, TC], BF16, isOutput=False)
    wq_d = nc.declare_dram_parameter("wq", [P, QKV_M, KH, P], BF16, isOutput=False)
    wo_d = nc.declare_dram_parameter("wo", [KV_DIM, HIDDEN],
                                     F8E4 if FP8_OUT else BF16, isOutput=False)
    srow_d = nc.declare_dram_parameter("srow", [1, 2 * TC], F32, isOutput=False)
    cos_d = nc.declare_dram_parameter("cosT", [32, T], BF16, isOutput=False)
    sin_d = nc.declare_dram_parameter("sinS", [64, T], BF16, isOutput=False)
    maska_d = nc.declare_dram_parameter("mask_a", [P, TC], BF16, isOutput=False)
    maskb_d = nc.declare_dram_parameter("mask_b", [P, TC], BF16, isOutput=False)
    out_d = nc.declare_dram_parameter("partial", [NCH, P, CD, TC], BF16, isOutput=True)

    with tile.TileContext(nc) as tc:
        _body(tc, nc, xb_d, wq_d, wo_d, srow_d, cos_d, sin_d,
              maska_d, maskb_d, out_d)
    nc.compile()
    return nc


def _body(tc, nc, xb_d, wq_d, wo_d, srow_d, cos_d, sin_d, maska_d, maskb_d, out_d):
    ATTN_DT = F8E4 if FP8_OUT else BF16
    ctx = contextlib.ExitStack()
    with ctx:
        const = ctx.enter_context(tc.tile_pool(name="const", bufs=1))
        xbf = ctx.enter_context(tc.tile_pool(name="xbf", bufs=2))
        qkvp = ctx.enter_context(tc.tile_pool(name="qkvp", bufs=7))
        qsp = ctx.enter_context(tc.tile_pool(name="qsp", bufs=2))
        krp = ctx.enter_context(tc.tile_pool(name="krp", bufs=2))
        kprevp = ctx.enter_context(tc.tile_pool(name="kprevp", bufs=2))
        tmpp = ctx.enter_context(tc.tile_pool(name="tmpp", bufs=2))
        vnatp = ctx.enter_context(tc.tile_pool(name="vnatp", bufs=10))
        expp = ctx.enter_context(tc.tile_pool(name="expp", bufs=4))
        probp = ctx.enter_context(tc.tile_pool(name="probp", bufs=6))
        smallp = ctx.enter_context(tc.tile_pool(name="smallp", bufs=3))
        rsbp = ctx.enter_context(tc.tile_pool(name="rsbp", bufs=3))
        attnp = ctx.enter_context(tc.tile_pool(name="attnp", bufs=2))
        outsb = ctx.enter_context(tc.tile_pool(name="outsb", bufs=3))

        ps_sc = ctx.enter_context(tc.tile_pool(name="ps_sc", bufs=4, space="PSUM"))
        ps_av = ctx.enter_context(tc.tile_pool(name="ps_av", bufs=2, space="PSUM"))
        ps_op = ctx.enter_context(tc.tile_pool(name="ps_op", bufs=2, space="PSUM"))

        # ---- prologue DMAs ----
        # scalar queue: wq m-slices then wo; sync queue: x chunk 0 pieces then
        # small consts.  First qkv matmul needs wq m-slice 0 + x piece 0.
        wq_sb = const.tile([P, QKV_M, KH, P], BF16)
        for m in (4, 0, 1, 2, 3):  # m=4 first: it is the first matmul emitted
            nc.scalar.dma_start(out=wq_sb[:, m], in_=wq_d[:, m])
        wo_sb = const.tile([P, 4, HIDDEN], ATTN_DT)
        for kk in range(4):
            nc.scalar.dma_start(out=wo_sb[:, kk, :], in_=wo_d[kk * P:(kk + 1) * P, :])

        cos_sb = const.tile([P, T], BF16)
        sin_sb = const.tile([P, T], BF16)
        srow_sb = const.tile([1, Q_MULT, P], F32)
        mask_a = const.tile([P, TC], BF16)
        mask_b = const.tile([P, TC], BF16)

        def emit_small_dmas():
            for b in range(4):
                nc.sync.dma_start(out=cos_sb[32 * b:32 * (b + 1), :], in_=cos_d[:])
            for b in range(2):
                nc.sync.dma_start(out=sin_sb[64 * b:64 * (b + 1), :], in_=sin_d[:])
            nc.sync.dma_start(out=srow_sb[:], in_=srow_d[:])
            nc.sync.dma_start(out=mask_a[:], in_=maska_d[:])
            nc.sync.dma_start(out=mask_b[:], in_=maskb_d[:])

        # ---------------- emitters ----------------

        def make_qkv(ch):
            """Returns (dma_fn, [m_fn x5], tiles); m_fn emits 23 matmuls+drain."""
            state = {}

            def dma_fn():
                xall = xbf.tile([P, KH, TC], BF16, tag="xk", name=f"xall_{ch}")
                for k0, k1 in ((0, 6), (6, 12), (12, 18), (18, KH)):
                    nc.sync.dma_start(out=xall[:, k0:k1, :],
                                      in_=xb_d[ch, :, k0:k1, :])
                state["x"] = xall

            tiles = [None] * QKV_M

            def make_m(m):
                def m_fn():
                    xall = state["x"]
                    pq = ps_sc.tile([P, TC], F32, tag="sc", name=f"pq_{ch}_{m}")
                    for k in range(KH):
                        nc.tensor.matmul(pq[:], wq_sb[:, m, k, :],
                                         xall[:, k, :],
                                         start=(k == 0), stop=(k == KH - 1))
                    qm = qkvp.tile([P, TC], BF16, tag="qkv", name=f"qm_{ch}_{m}")
                    nc.scalar.activation(qm[:], pq[:], AF.Copy)
                    tiles[m] = qm
                return m_fn

            return dma_fn, [make_m(m) for m in range(QKV_M)], tiles

        def make_rope(ch, qkv_tiles):
            """Per-m rope piece emitters (order 4,0,1,2,3); returns (qs, kr, fns)."""
            t0 = ch * TC
            qs = qsp.tile([64, Q_MULT, TC], BF16, tag="qs", name=f"qs_{ch}")
            kr = krp.tile([64, TC], BF16, tag="kr", name=f"kr_{ch}")

            def make_piece(m):
                def piece():
                    rows = P if m < 4 else HEAD_DIM
                    src = qkv_tiles[m]
                    sw = tmpp.tile([P, TC], BF16, tag="sw", bufs=1,
                                   name=f"sw_{ch}_{m}")
                    for b in range(rows // 64):
                        nc.vector.tensor_copy(sw[64 * b:64 * b + 32, :],
                                              src[64 * b + 32:64 * b + 64, :])
                        nc.vector.tensor_copy(sw[64 * b + 32:64 * b + 64, :],
                                              src[64 * b:64 * b + 32, :])
                    t1 = tmpp.tile([P, TC], BF16, tag="t1", bufs=1,
                                   name=f"t1_{ch}_{m}")
                    nc.vector.tensor_mul(t1[0:rows, :], src[0:rows, :],
                                         cos_sb[0:rows, t0:t0 + TC])
                    t2 = tmpp.tile([P, TC], BF16, tag="t2", bufs=1,
                                   name=f"t2_{ch}_{m}")
                    nc.vector.tensor_mul(t2[0:rows, :], sw[0:rows, :],
                                         sin_sb[0:rows, t0:t0 + TC])
                    if m < 4:
                        qr = tmpp.tile([P, TC], BF16, tag="qr", bufs=1,
                                       name=f"qr_{ch}_{m}")
                        nc.vector.tensor_add(qr[:], t1[:], t2[:])
                        nc.vector.tensor_copy(qs[:, 2 * m, :], qr[0:64, :])
                        nc.vector.tensor_copy(qs[:, 2 * m + 1, :], qr[64:P, :])
                    else:
                        nc.vector.tensor_add(kr[:], t1[0:64, :], t2[0:64, :])
                return piece

            return qs, kr, [make_piece(m) for m in (4, 0, 1, 2, 3)]

        def emit_vnat(ch, qkv_tiles):
            vn_tiles = []
            for tau in range(NT):
                vn = vnatp.tile([P, P], BF16, tag="vn", name=f"vn_{ch}_{tau}")
                nc.vector.memset(vn[:, HEAD_DIM:P], 0.0)
                nc.vector.memset(vn[:, HEAD_DIM:HEAD_DIM + 1], 1.0)
                nc.sync.dma_start(
                    out=vn[:, 0:HEAD_DIM],
                    in_=qkv_tiles[4][HEAD_DIM:P, tau * P:(tau + 1) * P],
                    transpose=True)
                vn_tiles.append(vn)
            return vn_tiles

        def emit_scores_tau(ch, tau, qs, kr, kprev):
            """Score matmuls + exps + mask-muls for both head groups."""
            tg = ch * NT + tau
            kprev_blk = (kr[:, (tau - 1) * P:tau * P] if tau > 0
                         else (kprev[:, :] if kprev is not None else None))
            prs = []
            ets = []
            for hg in range(2):
                qblk = qs[:, hg * 4:(hg + 1) * 4, tau * P:(tau + 1) * P]
                psc_a = ps_sc.tile([P, TC], F32, tag="sc", name=f"pa_{hg}")
                nc.tensor.matmul(psc_a[:], kr[:, tau * P:(tau + 1) * P], qblk,
                                 start=True, stop=True)
                et_a = expp.tile([P, TC], BF16, tag="et", name=f"ea_{hg}")
                nc.scalar.activation(et_a[:], psc_a[:], AF.Exp)
                psc_b = et_b = None
                if tg > 0:
                    psc_b = ps_sc.tile([P, TC], F32, tag="sc", name=f"pb_{hg}")
                    nc.tensor.matmul(psc_b[:], kprev_blk, qblk,
                                     start=True, stop=True)
                    et_b = expp.tile([P, TC], BF16, tag="et", name=f"eb_{hg}")
                    nc.scalar.activation(et_b[:], psc_b[:], AF.Exp)
                ets.append((et_a, et_b))
            for hg in range(2):
                et_a, et_b = ets[hg]
                pr_a = probp.tile([P, TC], BF16, tag="pr", name=f"fa_{hg}")
                nc.vector.tensor_mul(pr_a[:], et_a[:], mask_a[:])
                pr_b = None
                if et_b is not None:
                    pr_b = probp.tile([P, TC], BF16, tag="pr", name=f"fb_{hg}")
                    nc.vector.tensor_mul(pr_b[:], et_b[:], mask_b[:])
                prs.append((pr_a, pr_b))
            return prs

        def emit_pav_tau(ch, tau, prs, vn_tiles, attn_t, vnat_prev):
            """prob@v matmuls + normalize chain for both head groups."""
            tg = ch * NT + tau
            vprev_blk = vn_tiles[tau - 1] if tau > 0 else vnat_prev
            pavs = []
            for hg in range(2):
                pr_a, pr_b = prs[hg]
                pav = ps_av.tile([P, NT, P], F32, tag="av", name=f"pv_{hg}")
                nc.tensor.matmul(pav[:], vn_tiles[tau][:], pr_a[:],
                                 start=True, stop=(tg == 0))
                if tg > 0:
                    nc.tensor.matmul(pav[:], vprev_blk[:], pr_b[:],
                                     start=False, stop=True)
                pavs.append(pav)
            rrs = []
            for hg in range(2):
                den = smallp.tile([1, NT, P], F32, tag="den", name=f"dn_{hg}")
                nc.vector.scalar_tensor_tensor(
                    den[:], pavs[hg][64:65, :, :], 1.0 / ATTN_SCALE,
                    srow_sb[0:1, hg * 4:(hg + 1) * 4, :],
                    op0=ALU.mult, op1=ALU.add)
                rr = smallp.tile([1, NT, P], F32, tag="rr", name=f"rr_{hg}")
                nc.vector.reciprocal_approx_fast(rr[:], den[:])
                rrs.append(rr)
            rsbs = []
            for hg in range(2):
                rsb = rsbp.tile([HEAD_DIM, NT, P], F32, tag="rsb", name=f"rs_{hg}")
                nc.gpsimd.partition_broadcast(rsb[:], rrs[hg][:])
                rsbs.append(rsb)
            for hg in range(2):
                nc.vector.tensor_mul(
                    attn_t[hg * HEAD_DIM:(hg + 1) * HEAD_DIM, :, tau * P:(tau + 1) * P],
                    pavs[hg][0:HEAD_DIM, :, :], rsbs[hg][:])

        def make_op(ch, attn_t):
            """23 out-proj c-tile emitters for chunk ch; out DMAs batched x2."""
            state = {}

            def make_c(c, ci):
                cs = C_SIZES[c]

                def c_fn():
                    po = ps_op.tile([P, TC], F32, tag="op", name=f"po_{ch}_{c}")
                    if FP8_OUT:
                        for k2 in range(2):
                            nc.tensor.matmul(
                                po[0:cs, :],
                                wo_sb[:, 2 * k2:2 * k2 + 2, c * P:c * P + cs],
                                attn_t[:, 2 * k2:2 * k2 + 2, :],
                                start=(k2 == 0), stop=(k2 == 1),
                                perf_mode=mybir.MatmulPerfMode.DoubleRow)
                    else:
                        for kk in range(4):
                            nc.tensor.matmul(
                                po[0:cs, :], wo_sb[:, kk, c * P:c * P + cs],
                                attn_t[:, kk, :],
                                start=(kk == 0), stop=(kk == 3))
                    j = ci % 2
                    if j == 0:
                        state["ot"] = outsb.tile([P, 2, TC], BF16, tag="ot",
                                                 name=f"ot_{ch}_{c}")
                    ot = state["ot"]
                    # gpsimd cannot read PSUM; split drains scalar/vector
                    if ci % 2 == 0:
                        nc.scalar.activation(ot[0:cs, j, :], po[0:cs, :], AF.Copy)
                    else:
                        nc.vector.tensor_copy(ot[0:cs, j, :], po[0:cs, :])
                    if j == 1:
                        nc.sync.dma_start(out=out_d[ch, :, c - 1:c + 1, :],
                                          in_=ot[:, :, :])
                    elif c == CD - 1:
                        nc.sync.dma_start(out=out_d[ch, 0:cs, c:c + 1, :],
                                          in_=ot[0:cs, 0:1, :])
                return c_fn

            return [make_c(c, ci) for ci, c in enumerate(range(CD))]

        # ---------------- schedule ----------------
        # qkv m-tiles are emitted in order (4,0,1,2,3) so the k/v tile (m=4)
        # drains first: rope-k and the v transposes unblock early.
        M_ORDER = (4, 0, 1, 2, 3)

        dma0, qkv_m_fns, qkv_tiles_cur = make_qkv(0)
        dma0()
        emit_small_dmas()
        qs_c, kr_c, rope_c = make_rope(0, qkv_tiles_cur)
        vn_c = None
        for i, m in enumerate(M_ORDER):
            qkv_m_fns[m]()
            rope_c[i]()
            if m == 4:
                vn_c = emit_vnat(0, qkv_tiles_cur)

        kprev = None
        vnat_prev = None
        op_prev = []       # out-proj emitters for chunk ch-1

        for ch in range(NCH):
            attn_t = attnp.tile([P, NT, TC], ATTN_DT, tag="attn", name=f"attn_{ch}")

            if ch + 1 < NCH:
                dma_n, qkv_n_fns, qkv_tiles_next = make_qkv(ch + 1)
                dma_n()  # start the next x load before out-DMAs queue up
                qs_n, kr_n, rope_n = make_rope(ch + 1, qkv_tiles_next)
            else:
                qkv_n_fns, qkv_tiles_next = [], None
                qs_n = kr_n = rope_n = None
            vn_n = None

            # out-proj of ch-1 split around each tau's work
            opq = list(op_prev)

            def pop_ops(n):
                for fn in opq[:n]:
                    fn()
                del opq[:n]

            for tau in range(NT):
                pop_ops(3)
                prs = emit_scores_tau(ch, tau, qs_c, kr_c, kprev)
                if qkv_n_fns:
                    qkv_n_fns[M_ORDER[tau]]()
                else:
                    pop_ops(3)
                emit_pav_tau(ch, tau, prs, vn_c, attn_t, vnat_prev)
                if rope_n is not None:
                    rope_n[tau]()
                    if tau == 0:
                        vn_n = emit_vnat(ch + 1, qkv_tiles_next)
                pop_ops(3 if tau < NT - 1 else 23)
                if tau == NT - 1 and qkv_n_fns:
                    qkv_n_fns[M_ORDER[4]]()
                    rope_n[4]()

            kprev_t = kprevp.tile([HEAD_DIM, P], BF16, tag="kp", name=f"kp_{ch}")
            nc.vector.tensor_copy(kprev_t[:], kr_c[:, TC - P:TC])
            kprev = kprev_t
            vnat_prev = vn_c[NT - 1]
            op_prev = make_op(ch, attn_t)
            qs_c, kr_c, vn_c = qs_n, kr_n, vn_n
            qkv_tiles_cur = qkv_tiles_next

        # epilogue: out-proj of the last chunk
        for fn in op_prev:
            fn()


# ------------------------- entry point -------------------------

def _get_program():
    if "nc" not in _CACHE:
        _CACHE["nc"] = build_program()
    return _CACHE["nc"]


def run_cores(inputs, trace=False):
    per_core = _host_prepare(**inputs)
    nc = _get_program()
    res = bass_utils.run_bass_kernel_spmd(
        nc, per_core, core_ids=list(range(N_CORES)), trace=trace,
    )
    return res


def kernel(**inputs):
    res = run_cores(inputs)
    acc = np.zeros((HIDDEN, T), dtype=np.float64)
    for r in res.results:
        acc += _partial_to_full(r["partial"])
    out = acc.T + inputs["x"][0].astype(np.float64) + inputs["b_out"][None, :].astype(np.float64)
    return out[None].astype(np.float32)


# revision 15
# speedup vs baseline: 2.3226x; 1.0353x over previous
"""Trainium2 Bass kernel for nn_AttentionBlock (sliding-window attention block).

Distribution: tensor-parallel over the 8 KV head groups (one group per core).
Each core computes qkv^T for its group (640 rows x 2048 tokens), windowed
attention for its 8 q-heads, and a partial output projection; host sums the
8 partials and adds x + b_out.

Device layout is feature-major: features on partitions, tokens on the free
dim.  Host pre-transposes and pre-rmsnorms x (bf16) and the weights.

Key structure (v3):
- x shipped pre-normalized (x*sqrt(H)/s); bias enters via a ones u-row.
- xb chunk-contiguous in DRAM ([NCH, P, KH, TC]), loaded in 4 pieces per
  chunk so the qkv k-loop can start before the full chunk lands.
- wq shipped partition-major ([P, M, KH, 128]) so each m-slice loads with
  one fat DMA; k-tiles zero-padded to 128 rows.
- softmax denominator: sink added on DVE (scalar_tensor_tensor), reciprocal
  on DVE, partition-broadcast on GpSimd -- no PE rank-1 matmuls.
- attention output tile is [128, 4, TC] with head h -> (kk=h%4, half=h//4)
  so the normalize multiply writes it directly (no layout copies); w_out is
  host-reordered to match.
- per-tau PE queue: [scores][op 3c][qkv m][pav][op 3c] so the PE never
  waits on the exp/mask/normalize chains; out-proj of chunk ch-1 and qkv of
  ch+1 fill all gaps.  GpSimd runs only partition_broadcast + rope copies
  (it is slow and must stay off the critical path).
- optional fp8 out-projection (DoubleRow) controlled by FP8_OUT.
"""

import contextlib
import math

import numpy as np
import ml_dtypes

import concourse.bass as bass
import concourse.mybir as mybir
import concourse.tile as tile
from concourse import bacc, bass_utils

# ---- problem config (hardcoded from the reference) ----
HIDDEN = 2880
HEAD_DIM = 64
N_HEADS = 64
N_KV = 8
Q_MULT = N_HEADS // N_KV  # 8
SLIDING_WINDOW = 128
ROPE_BASE = 150000.0
ROPE_SCALE = 32.0
NTK_ALPHA = 1.0
NTK_BETA = 32.0
INIT_CTX = 4096
RMS_EPS = 1e-5
SM_SCALE = 1.0 / math.sqrt(HEAD_DIM)
Q_DIM = N_HEADS * HEAD_DIM  # 4096
KV_DIM = N_KV * HEAD_DIM  # 512
B, T = 1, 2048

N_CORES = 8
P = 128
TC = 512  # token chunk
NCH = T // TC  # 4 chunks
NT = TC // P  # 4 token blocks per chunk
KH = 23  # hidden tiles: 23x128 (last zero-padded, carries the bias u-row)
KHP = KH * P  # 2944
QKV_ROWS = Q_MULT * HEAD_DIM + 2 * HEAD_DIM  # 640
QKV_M = QKV_ROWS // P  # 5
CD = 23  # out-proj c tiles: 22x128 + 1x64
C_SIZES = [128] * 22 + [64]

F32 = mybir.dt.float32
BF16 = mybir.dt.bfloat16
F8E4 = mybir.dt.float8e4
AF = mybir.ActivationFunctionType
ALU = mybir.AluOpType

FP8_OUT = True            # fp8 (DoubleRow) out-projection
ATTN_SCALE = 8.0 if FP8_OUT else 1.0
WO_SCALE = 64.0 if FP8_OUT else 1.0

_CACHE = {}


# ------------------------- host-side preparation -------------------------

def _rope_tables():
    d_half = HEAD_DIM // 2
    freq = ROPE_BASE ** (np.arange(0, HEAD_DIM, 2, dtype=np.float64) / HEAD_DIM)
    concentration = 0.1 * math.log(ROPE_SCALE) + 1.0
    low = d_half * math.log(INIT_CTX / (NTK_BETA * 2 * math.pi)) / math.log(ROPE_BASE)
    high = d_half * math.log(INIT_CTX / (NTK_ALPHA * 2 * math.pi)) / math.log(ROPE_BASE)
    interpolation = 1.0 / (ROPE_SCALE * freq)
    extrapolation = 1.0 / freq
    ramp = (np.arange(d_half, dtype=np.float64) - low) / (high - low)
    mask = 1.0 - np.clip(ramp, 0.0, 1.0)
    inv_freq = interpolation * (1.0 - mask) + extrapolation * mask
    pos = np.arange(T, dtype=np.float64)
    angles = pos[:, None] * inv_freq[None, :]  # [T, 32]
    cos = (np.cos(angles) * concentration).astype(np.float32)
    sin = (np.sin(angles) * concentration).astype(np.float32)
    return cos.T.copy(), sin.T.copy()  # [32, T]


def _perm64():
    # evens then odds within a 64-dim head
    return np.concatenate([np.arange(0, 64, 2), np.arange(1, 64, 2)])


def _host_prepare(x, norm_scale, w_qkv, b_qkv, sinks, w_out, b_out):
    x64 = x[0].astype(np.float64)
    s = np.sqrt((x64 * x64).sum(axis=1) + HIDDEN * RMS_EPS)  # [2048]
    xn = x64 * (math.sqrt(HIDDEN) / s)[:, None]  # pre-rmsnormed, ~N(0,1)
    xf = np.zeros((KHP, T), dtype=np.float32)
    xf[:HIDDEN] = xn.T
    xf[HIDDEN] = 1.0  # u row (bias)
    # chunk-contiguous: [NCH, P, KH, TC]
    xb = np.ascontiguousarray(
        xf.reshape(KH, P, NCH, TC).transpose(2, 1, 0, 3)).astype(ml_dtypes.bfloat16)

    w_eff = (w_qkv * norm_scale[None, :]).astype(np.float64)
    b_eff = b_qkv.astype(np.float64).copy()
    # fold softmax scale into q rows (rope is a rotation; scale commutes)
    w_eff[:Q_DIM] *= SM_SCALE
    b_eff[:Q_DIM] *= SM_SCALE

    perm = _perm64()
    cosT, sinT = _rope_tables()
    sin_signed = np.concatenate([-sinT, sinT], axis=0)  # [64, T]

    # band masks for the two score blocks, repeated over 4 heads
    pidx = np.arange(P)[:, None]
    jidx = np.arange(P)[None, :]
    mask_a = np.tile((jidx >= pidx), (1, 4)).astype(ml_dtypes.bfloat16)  # [128, 512]
    mask_b = np.tile((jidx <= pidx), (1, 4)).astype(ml_dtypes.bfloat16)

    per_core = []
    for g in range(N_CORES):
        rows = []
        for h in range(Q_MULT):  # q heads of this group, rope-permuted
            base = (g * Q_MULT + h) * HEAD_DIM
            rows.append(base + perm)
        rows.append(Q_DIM + g * HEAD_DIM + perm)  # k head, rope-permuted
        rows.append(Q_DIM + KV_DIM + g * HEAD_DIM + np.arange(HEAD_DIM))  # v natural
        rows = np.concatenate(rows)
        wq_g = np.zeros((KHP, QKV_ROWS), dtype=np.float64)
        wq_g[:HIDDEN] = w_eff[rows].T
        wq_g[HIDDEN] = b_eff[rows]
        # partition-major for fat per-m DMAs: [P, M, KH, 128]
        wq_g = np.ascontiguousarray(
            wq_g.reshape(KH, P, QKV_M, P).transpose(1, 2, 0, 3)
        ).astype(ml_dtypes.bfloat16)

        # w_out columns for this group, reordered so attn row r of kk-tile j is:
        #   rows 0:64  -> head j      (hg0)
        #   rows 64:128-> head 4 + j  (hg1)
        wo_local = w_out[:, g * KV_DIM:(g + 1) * KV_DIM].T  # [512, 2880]
        order = []
        for kk in range(4):
            order.extend(range(kk * 64, kk * 64 + 64))
            order.extend(range((4 + kk) * 64, (4 + kk) * 64 + 64))
        wo_g = np.ascontiguousarray(wo_local[order])  # [512, 2880]
        if FP8_OUT:
            wo_g = np.clip(wo_g * WO_SCALE, -240, 240).astype(ml_dtypes.float8_e4m3)
        else:
            wo_g = wo_g.astype(ml_dtypes.bfloat16)

        sexp = np.exp2(sinks[g * Q_MULT:(g + 1) * Q_MULT]).astype(np.float64)
        srow = np.repeat(sexp / ATTN_SCALE, P).reshape(1, -1).astype(np.float32)
        per_core.append({
            "xb": xb,
            "wq": wq_g,
            "wo": wo_g,
            "srow": srow,  # [1, 1024] f32, sink/ATTN_SCALE per head
            "cosT": cosT.astype(ml_dtypes.bfloat16),
            "sinS": sin_signed.astype(ml_dtypes.bfloat16),
            "mask_a": mask_a,
            "mask_b": mask_b,
        })
    return per_core


def _partial_to_full(partial):
    """Device partial [NCH, P, CD, TC] -> [HIDDEN, T] float64 (unscaled)."""
    p = np.asarray(partial, dtype=np.float64)
    # row c*P + p of chunk ch lives at partial[ch, p, c, :]
    out = p.transpose(2, 1, 0, 3).reshape(CD * P, T)[:HIDDEN]
    return out / (ATTN_SCALE * WO_SCALE)


# ------------------------- device program -------------------------

def build_program():
    nc = bacc.Bacc(None, target_bir_lowering=False)

    xb_d = nc.declare_dram_parameter("xb", [NCH, P, KH, TC], BF16, isOutput=False)
    wq_d = nc.declare_dram_parameter("wq", [P, QKV_M, KH, P], BF16, isOutput=False)
    wo_d = nc.declare_dram_parameter("wo", [KV_DIM, HIDDEN],
                                     F8E4 if FP8_OUT else BF16, isOutput=False)
    srow_d = nc.declare_dram_parameter("srow", [1, 2 * TC], F32, isOutput=False)
    cos_d = nc.declare_dram_parameter("cosT", [32, T], BF16, isOutput=False)
    sin_d = nc.declare_dram_parameter("sinS", [64, T], BF16, isOutput=False)
    maska_d = nc.declare_dram_parameter("mask_a", [P, TC], BF16, isOutput=False)
    maskb_d = nc.declare_dram_parameter("mask_b", [P, TC], BF16, isOutput=False)
    out_d = nc.declare_dram_parameter("partial", [NCH, P, CD, TC], BF16, isOutput=True)

    with tile.TileContext(nc) as tc:
        _body(tc, nc, xb_d, wq_d, wo_d, srow_d, cos_d, sin_d,
              maska_d, maskb_d, out_d)
    nc.compile()
    return nc


def _body(tc, nc, xb_d, wq_d, wo_d, srow_d, cos_d, sin_d, maska_d, maskb_d, out_d):
    ATTN_DT = F8E4 if FP8_OUT else BF16
    ctx = contextlib.ExitStack()
    with ctx:
        const = ctx.enter_context(tc.tile_pool(name="const", bufs=1))
        xbf = ctx.enter_context(tc.tile_pool(name="xbf", bufs=2))
        qkvp = ctx.enter_context(tc.tile_pool(name="qkvp", bufs=7))
        qsp = ctx.enter_context(tc.tile_pool(name="qsp", bufs=2))
        krp = ctx.enter_context(tc.tile_pool(name="krp", bufs=2))
        kprevp = ctx.enter_context(tc.tile_pool(name="kprevp", bufs=2))
        tmpp = ctx.enter_context(tc.tile_pool(name="tmpp", bufs=2))
        vnatp = ctx.enter_context(tc.tile_pool(name="vnatp", bufs=10))
        expp = ctx.enter_context(tc.tile_pool(name="expp", bufs=4))
        probp = ctx.enter_context(tc.tile_pool(name="probp", bufs=6))
        smallp = ctx.enter_context(tc.tile_pool(name="smallp", bufs=3))
        rsbp = ctx.enter_context(tc.tile_pool(name="rsbp", bufs=3))
        attnp = ctx.enter_context(tc.tile_pool(name="attnp", bufs=2))
        outsb = ctx.enter_context(tc.tile_pool(name="outsb", bufs=3))

        ps_sc = ctx.enter_context(tc.tile_pool(name="ps_sc", bufs=4, space="PSUM"))
        ps_av = ctx.enter_context(tc.tile_pool(name="ps_av", bufs=2, space="PSUM"))
        ps_op = ctx.enter_context(tc.tile_pool(name="ps_op", bufs=2, space="PSUM"))

        # ---- prologue DMAs ----
        # scalar queue: wq m-slices then wo; sync queue: x chunk 0 pieces then
        # small consts.  First qkv matmul needs wq m-slice 0 + x piece 0.
        wq_sb = const.tile([P, QKV_M, KH, P], BF16)
        for m in (4, 0, 1, 2, 3):  # m=4 first: it is the first matmul emitted
            nc.scalar.dma_start(out=wq_sb[:, m], in_=wq_d[:, m])
        wo_sb = const.tile([P, 4, HIDDEN], ATTN_DT)
        for kk in range(4):
            nc.scalar.dma_start(out=wo_sb[:, kk, :], in_=wo_d[kk * P:(kk + 1) * P, :])

        cos_sb = const.tile([P, T], BF16)
        sin_sb = const.tile([P, T], BF16)
        srow_sb = const.tile([1, Q_MULT, P], F32)
        mask_a = const.tile([P, TC], BF16)
        mask_b = const.tile([P, TC], BF16)

        def emit_small_dmas():
            for b in range(4):
                nc.sync.dma_start(out=cos_sb[32 * b:32 * (b + 1), :], in_=cos_d[:])
            for b in range(2):
                nc.sync.dma_start(out=sin_sb[64 * b:64 * (b + 1), :], in_=sin_d[:])
            nc.sync.dma_start(out=srow_sb[:], in_=srow_d[:])
            nc.sync.dma_start(out=mask_a[:], in_=maska_d[:])
            nc.sync.dma_start(out=mask_b[:], in_=maskb_d[:])

        # ---------------- emitters ----------------

        def make_qkv(ch):
            """Returns (dma_fn, [m_fn x5], tiles); m_fn emits 23 matmuls+drain."""
            state = {}

            def dma_fn():
                xall = xbf.tile([P, KH, TC], BF16, tag="xk", name=f"xall_{ch}")
                for k0, k1 in ((0, 6), (6, 12), (12, 18), (18, KH)):
                    nc.sync.dma_start(out=xall[:, k0:k1, :],
                                      in_=xb_d[ch, :, k0:k1, :])
                state["x"] = xall

            tiles = [None] * QKV_M

            def make_m(m):
                def m_fn():
                    xall = state["x"]
                    pq = ps_sc.tile([P, TC], F32, tag="sc", name=f"pq_{ch}_{m}")
                    for k in range(KH):
                        nc.tensor.matmul(pq[:], wq_sb[:, m, k, :],
                                         xall[:, k, :],
                                         start=(k == 0), stop=(k == KH - 1))
                    qm = qkvp.tile([P, TC], BF16, tag="qkv", name=f"qm_{ch}_{m}")
                    nc.scalar.activation(qm[:], pq[:], AF.Copy)
                    tiles[m] = qm
                return m_fn

            return dma_fn, [make_m(m) for m in range(QKV_M)], tiles

        def make_rope(ch, qkv_tiles):
            """Per-m rope piece emitters (order 4,0,1,2,3); returns (qs, kr, fns)."""
            t0 = ch * TC
            qs = qsp.tile([64, Q_MULT, TC], BF16, tag="qs", name=f"qs_{ch}")
            kr = krp.tile([64, TC], BF16, tag="kr", name=f"kr_{ch}")

            def make_piece(m):
                def piece():
                    rows = P if m < 4 else HEAD_DIM
                    src = qkv_tiles[m]
                    sw = tmpp.tile([P, TC], BF16, tag="sw", bufs=1,
                                   name=f"sw_{ch}_{m}")
                    for b in range(rows // 64):
                        nc.vector.tensor_copy(sw[64 * b:64 * b + 32, :],
                                              src[64 * b + 32:64 * b + 64, :])
                        nc.vector.tensor_copy(sw[64 * b + 32:64 * b + 64, :],
                                              src[64 * b:64 * b + 32, :])
                    t1 = tmpp.tile([P, TC], BF16, tag="t1", bufs=1,
                                   name=f"t1_{ch}_{m}")
                    nc.vector.tensor_mul(t1[0:rows, :], src[0:rows, :],
                                         cos_sb[0:rows, t0:t0 + TC])
                    t2 = tmpp.tile([P, TC], BF16, tag="t2", bufs=1,
                                   name=f"t2_{ch}_{m}")
                    nc.vector.tensor_mul(t2[0:rows, :], sw[0:rows, :],
                                         sin_sb[0:rows, t0:t0 + TC])
                    if m < 4:
                        qr = tmpp.tile([P, TC], BF16, tag="qr", bufs=1,
                                       name=f"qr_{ch}_{m}")
                        nc.vector.tensor_add(qr[:], t1[:], t2[:])
                        nc.vector.tensor_copy(qs[:, 2 * m, :], qr[0:64, :])
                        nc.vector.tensor_copy(qs[:, 2 * m + 1, :], qr[64:P, :])
                    else:
                        nc.vector.tensor_add(kr[:], t1[0:64, :], t2[0:64, :])
                return piece

            return qs, kr, [make_piece(m) for m in (4, 0, 1, 2, 3)]

        def emit_vnat(ch, qkv_tiles):
            vn_tiles = []
            for tau in range(NT):
                vn = vnatp.tile([P, P], BF16, tag="vn", name=f"vn_{ch}_{tau}")
                nc.vector.memset(vn[:, HEAD_DIM:P], 0.0)
                nc.vector.memset(vn[:, HEAD_DIM:HEAD_DIM + 1], 1.0)
                nc.sync.dma_start(
                    out=vn[:, 0:HEAD_DIM],
                    in_=qkv_tiles[4][HEAD_DIM:P, tau * P:(tau + 1) * P],
                    transpose=True)
                vn_tiles.append(vn)
            return vn_tiles

        def emit_scores_tau(ch, tau, qs, kr, kprev):
            """Score matmuls + exps + mask-muls for both head groups."""
            tg = ch * NT + tau
            kprev_blk = (kr[:, (tau - 1) * P:tau * P] if tau > 0
                         else (kprev[:, :] if kprev is not None else None))
            prs = []
            ets = []
            for hg in range(2):
                qblk = qs[:, hg * 4:(hg + 1) * 4, tau * P:(tau + 1) * P]
                psc_a = ps_sc.tile([P, TC], F32, tag="sc", name=f"pa_{hg}")
                nc.tensor.matmul(psc_a[:], kr[:, tau * P:(tau + 1) * P], qblk,
                                 start=True, stop=True)
                et_a = expp.tile([P, TC], BF16, tag="et", name=f"ea_{hg}")
                nc.scalar.activation(et_a[:], psc_a[:], AF.Exp)
                psc_b = et_b = None
                if tg > 0:
                    psc_b = ps_sc.tile([P, TC], F32, tag="sc", name=f"pb_{hg}")
                    nc.tensor.matmul(psc_b[:], kprev_blk, qblk,
                                     start=True, stop=True)
                    et_b = expp.tile([P, TC], BF16, tag="et", name=f"eb_{hg}")
                    nc.scalar.activation(et_b[:], psc_b[:], AF.Exp)
                ets.append((et_a, et_b))
            for hg in range(2):
                et_a, et_b = ets[hg]
                pr_a = probp.tile([P, TC], BF16, tag="pr", name=f"fa_{hg}")
                nc.vector.tensor_mul(pr_a[:], et_a[:], mask_a[:])
                pr_b = None
                if et_b is not None:
                    pr_b = probp.tile([P, TC], BF16, tag="pr", name=f"fb_{hg}")
                    nc.vector.tensor_mul(pr_b[:], et_b[:], mask_b[:])
                prs.append((pr_a, pr_b))
            return prs

        def emit_pav_tau(ch, tau, prs, vn_tiles, attn_t, vnat_prev):
            """prob@v matmuls + normalize chain for both head groups."""
            tg = ch * NT + tau
            vprev_blk = vn_tiles[tau - 1] if tau > 0 else vnat_prev
            pavs = []
            for hg in range(2):
                pr_a, pr_b = prs[hg]
                pav = ps_av.tile([P, NT, P], F32, tag="av", name=f"pv_{hg}")
                nc.tensor.matmul(pav[:], vn_tiles[tau][:], pr_a[:],
                                 start=True, stop=(tg == 0))
                if tg > 0:
                    nc.tensor.matmul(pav[:], vprev_blk[:], pr_b[:],
                                     start=False, stop=True)
                pavs.append(pav)
            rrs = []
            for hg in range(2):
                den = smallp.tile([1, NT, P], F32, tag="den", name=f"dn_{hg}")
                nc.vector.scalar_tensor_tensor(
                    den[:], pavs[hg][64:65, :, :], 1.0 / ATTN_SCALE,
                    srow_sb[0:1, hg * 4:(hg + 1) * 4, :],
                    op0=ALU.mult, op1=ALU.add)
                rr = smallp.tile([1, NT, P], F32, tag="rr", name=f"rr_{hg}")
                nc.vector.reciprocal_approx_fast(rr[:], den[:])
                rrs.append(rr)
            rsbs = []
            for hg in range(2):
                rsb = rsbp.tile([HEAD_DIM, NT, P], F32, tag="rsb", name=f"rs_{hg}")
                nc.gpsimd.partition_broadcast(rsb[:], rrs[hg][:])
                rsbs.append(rsb)
            for hg in range(2):
                nc.vector.tensor_mul(
                    attn_t[hg * HEAD_DIM:(hg + 1) * HEAD_DIM, :, tau * P:(tau + 1) * P],
                    pavs[hg][0:HEAD_DIM, :, :], rsbs[hg][:])

        def make_op(ch, attn_t):
            """23 out-proj c-tile emitters for chunk ch; out DMAs batched x2."""
            state = {}

            def make_c(c, ci):
                cs = C_SIZES[c]

                def c_fn():
                    po = ps_op.tile([P, TC], F32, tag="op", name=f"po_{ch}_{c}")
                    if FP8_OUT:
                        for k2 in range(2):
                            nc.tensor.matmul(
                                po[0:cs, :],
                                wo_sb[:, 2 * k2:2 * k2 + 2, c * P:c * P + cs],
                                attn_t[:, 2 * k2:2 * k2 + 2, :],
                                start=(k2 == 0), stop=(k2 == 1),
                                perf_mode=mybir.MatmulPerfMode.DoubleRow)
                    else:
                        for kk in range(4):
                            nc.tensor.matmul(
                                po[0:cs, :], wo_sb[:, kk, c * P:c * P + cs],
                                attn_t[:, kk, :],
                                start=(kk == 0), stop=(kk == 3))
                    j = ci % 2
                    if j == 0:
                        state["ot"] = outsb.tile([P, 2, TC], BF16, tag="ot",
                                                 name=f"ot_{ch}_{c}")
                    ot = state["ot"]
                    # gpsimd cannot read PSUM; split drains scalar/vector
                    if ci % 2 == 0:
                        nc.scalar.activation(ot[0:cs, j, :], po[0:cs, :], AF.Copy)
                    else:
                        nc.vector.tensor_copy(ot[0:cs, j, :], po[0:cs, :])
                    if j == 1:
                        nc.sync.dma_start(out=out_d[ch, :, c - 1:c + 1, :],
                                          in_=ot[:, :, :])
                    elif c == CD - 1:
                        nc.sync.dma_start(out=out_d[ch, 0:cs, c:c + 1, :],
                                          in_=ot[0:cs, 0:1, :])
                return c_fn

            return [make_c(c, ci) for ci, c in enumerate(range(CD))]

        # ---------------- schedule ----------------
        # qkv m-tiles are emitted in order (4,0,1,2,3) so the k/v tile (m=4)
        # drains first: rope-k and the v transposes unblock early.
        M_ORDER = (4, 0, 1, 2, 3)

        dma0, qkv_m_fns, qkv_tiles_cur = make_qkv(0)
        dma0()
        emit_small_dmas()
        qs_c, kr_c, rope_c = make_rope(0, qkv_tiles_cur)
        vn_c = None
        for i, m in enumerate(M_ORDER):
            qkv_m_fns[m]()
            rope_c[i]()
            if m == 4:
                vn_c = emit_vnat(0, qkv_tiles_cur)

        kprev = None
        vnat_prev = None
        op_prev = []       # out-proj emitters for chunk ch-1

        for ch in range(NCH):
            attn_t = attnp.tile([P, NT, TC], ATTN_DT, tag="attn", name=f"attn_{ch}")

            if ch + 1 < NCH:
                dma_n, qkv_n_fns, qkv_tiles_next = make_qkv(ch + 1)
                dma_n()  # start the next x load before out-DMAs queue up
                qs_n, kr_n, rope_n = make_rope(ch + 1, qkv_tiles_next)
            else:
                qkv_n_fns, qkv_tiles_next = [], None
                qs_n = kr_n = rope_n = None
            vn_n = None

            # out-proj of ch-1 split around each tau's work
            opq = list(op_prev)

            def pop_ops(n):
                for fn in opq[:n]:
                    fn()
                del opq[:n]

            kprev_next = None
            for tau in range(NT):
                prs = emit_scores_tau(ch, tau, qs_c, kr_c, kprev)
                if tau == 0:
                    # kprev for the NEXT window; kr_c was roped last window
                    kprev_next = kprevp.tile([HEAD_DIM, P], BF16, tag="kp",
                                             name=f"kp_{ch}")
                    nc.vector.tensor_copy(kprev_next[:], kr_c[:, TC - P:TC])
                pop_ops(3)
                # qkv slots lag one tau so the x DMA has ~10us of lead time
                if qkv_n_fns and tau >= 1:
                    qkv_n_fns[M_ORDER[tau - 1]]()
                else:
                    pop_ops(3)
                emit_pav_tau(ch, tau, prs, vn_c, attn_t, vnat_prev)
                if rope_n is not None and tau >= 1:
                    rope_n[tau - 1]()
                    if tau == 1:
                        vn_n = emit_vnat(ch + 1, qkv_tiles_next)
                pop_ops(3 if tau < NT - 1 else 23)
                if tau == NT - 1 and qkv_n_fns:
                    qkv_n_fns[M_ORDER[3]]()
                    rope_n[3]()
                    qkv_n_fns[M_ORDER[4]]()
                    rope_n[4]()

            kprev = kprev_next
            vnat_prev = vn_c[NT - 1]
            op_prev = make_op(ch, attn_t)
            qs_c, kr_c, vn_c = qs_n, kr_n, vn_n
            qkv_tiles_cur = qkv_tiles_next

        # epilogue: out-proj of the last chunk
        for fn in op_prev:
            fn()


# ------------------------- entry point -------------------------

def _get_program():
    if "nc" not in _CACHE:
        _CACHE["nc"] = build_program()
    return _CACHE["nc"]


def run_cores(inputs, trace=False):
    per_core = _host_prepare(**inputs)
    nc = _get_program()
    res = bass_utils.run_bass_kernel_spmd(
        nc, per_core, core_ids=list(range(N_CORES)), trace=trace,
    )
    return res


def kernel(**inputs):
    res = run_cores(inputs)
    acc = np.zeros((HIDDEN, T), dtype=np.float64)
    for r in res.results:
        acc += _partial_to_full(r["partial"])
    out = acc.T + inputs["x"][0].astype(np.float64) + inputs["b_out"][None, :].astype(np.float64)
    return out[None].astype(np.float32)
